# revision 12
# baseline (speedup 1.0000x reference)
"""Trainium2 Bass kernel for nn_EntropyModel (minGRU LM).

Strategy (8 NeuronCores, data-parallel over batch B=8, one sample per core):

  - Residual stream kept TRANSPOSED on device: hT[d, s] (d on partitions,
    s along free dim), because the minGRU recurrence is computed with the
    DVE `tensor_tensor_scan` instruction (state = a*state + v along the
    free dim, fp32 internal state) which needs lanes on partitions and
    time along free.
  - Layer 1 is token-lookup: rms/hg/nonlinearities of layer 1 depend only
    on the token id (vocab=256), so a1/v1 (scan coefficients/values) and
    h0 (embedding) are precomputed on host as tables and gathered per
    token; the device only runs the scan + out-projection for layer 1.
  - minGRU math:  a = sigmoid(-gate) = 1 - z,  v = z * g(hidden) with
    g(x) = max(x + 0.5, sigmoid(x))  (exact identity for the reference's
    where(x>=0, x+0.5, sigmoid(x))).
    Device computes a directly via sigmoid(-gate) (ACT scale=-1) and
    vneg = (a - 1) * g = -v in ONE scalar_tensor_tensor op; the scan then
    produces -h_gru and the sign is folded into W_out host-side.
  - rmsnorm weight folded into W_hg host-side; layernorm w/b folded into
    the vocab projection: logits = rstd*(W2.T h - mu*colsum(W2)) + b2.
  - b_out rides the out-projection PSUM eviction as the ACT bias operand
    (out = Copy(ps + b_out)); the residual add stays an identity matmul
    inside the PSUM accumulation.
  - Per-token sums over d (rms sumsq, LN mean/meansq) via ones-vector
    matmuls on the PE; per-free broadcast of r[s] via K=1 rank-1 matmuls.
  - Engine balance: squares for the stats run on the (otherwise idle)
    GpSimd engine as fp8 tiles; the final logits corrections also run on
    GpSimd after an ACT PSUM eviction; all rsqrts use the DVE Newton
    iteration so the ACT sigmoid table stays resident (ACT table reloads
    cost 1.3us each).
  - Stats for layer li+1 are emitted inside layer li's chunk loop (chunk
    c-2 after out-proj of c-1) so layer boundaries don't stall the PE;
    the final-LN stats for the first half ride inside layer L-1's loop.

Matmul/bf16 dtype knobs below (MM_*: matmul operands, SC_*: scan operands).
"""

import os
import numpy as np
import ml_dtypes

V, D, L, B, S = 256, 512, 4, 8, 4096
EPS_RMS = 1e-5
EPS_LN = 1e-5
P = 128
KT = D // P            # 4 d-tiles of 128
ET = 2 * D // P        # 8 e-tiles for the hidden/gate projection
SC = 1024              # s-chunk for working tiles
NSC = S // SC
N_CORES = 8

# dtype knobs: "bf16" or "f32r" for matmul operands, "bf16"/"f32" for scan
MM_KIND = os.environ.get("EM_MM_KIND", "bf16")
SC_KIND = os.environ.get("EM_SC_KIND", "bf16")

_cache = {}


def _np_dt(kind):
    return np.float32 if kind in ("f32", "f32r") else ml_dtypes.bfloat16


def _build_nc():
    import concourse.bass as bass  # noqa: F401
    import concourse.bacc as bacc
    import concourse.mybir as mybir
    import concourse.tile as tile
    from contextlib import ExitStack

    AL = mybir.AluOpType
    AF = mybir.ActivationFunctionType
    F32 = mybir.dt.float32
    I32 = mybir.dt.int32
    F8 = mybir.dt.float8e4
    MM = {"bf16": mybir.dt.bfloat16, "f32r": mybir.dt.float32r}[MM_KIND]
    SCD = {"bf16": mybir.dt.bfloat16, "f32": mybir.dt.float32}[SC_KIND]

    nc = bacc.Bacc()

    # layer-1 streams packed chunk-major: [c, p, k*SC+j] = orig[k*128+p, c*SC+j]
    # so each chunk is ONE ~1MB DMA (descriptor overhead amortized)
    d_h0T = nc.dram_tensor("h0T", [NSC, P, KT * SC], MM, kind="ExternalInput")
    d_a1T = nc.dram_tensor("a1T", [NSC, P, KT * SC], SCD, kind="ExternalInput")
    d_v1T = nc.dram_tensor("v1T", [NSC, P, KT * SC], SCD, kind="ExternalInput")
    d_W1 = nc.dram_tensor("W1", [L - 1, KT, P, 2 * D], MM, kind="ExternalInput")
    d_WoT = nc.dram_tensor("WoT", [L, KT, P, D], MM, kind="ExternalInput")
    d_W2 = nc.dram_tensor("W2", [KT, P, V], MM, kind="ExternalInput")
    d_bout = nc.dram_tensor("bout", [L, KT, P, 1], F32, kind="ExternalInput")
    d_b2rep = nc.dram_tensor("b2rep", [P, V], F32, kind="ExternalInput")
    d_cwrep = nc.dram_tensor("cwrep", [P, V], F32, kind="ExternalInput")
    d_onessel = nc.dram_tensor("onessel", [P, 16], MM, kind="ExternalInput")
    d_onesk = nc.dram_tensor("onesk", [P, 1], MM, kind="ExternalInput")
    d_onesr = nc.dram_tensor("onesr", [1, P], MM, kind="ExternalInput")
    d_ident = nc.dram_tensor("ident", [P, P], MM, kind="ExternalInput")
    d_identf = nc.dram_tensor("identf", [4, 4], F32, kind="ExternalInput")
    # output laid out [group, partition, 4*V]: token s = g*512 + j*128 + p
    # lives at out[g, p, j*V:(j+1)*V] so a group of 4 token-tiles is one
    # contiguous [128, 1024] DMA; the host un-permutes.
    d_out = nc.dram_tensor("out", [S // (4 * P), P, 4 * V], F32,
                           kind="ExternalOutput")

    with ExitStack() as ctx:
        tc = ctx.enter_context(tile.TileContext(nc))
        consts = ctx.enter_context(tc.tile_pool(name="consts", bufs=1))
        hpool = ctx.enter_context(tc.tile_pool(name="hpool", bufs=1))
        work = ctx.enter_context(tc.tile_pool(name="work", bufs=2))
        small = ctx.enter_context(tc.tile_pool(name="small", bufs=1))
        ps_mm = ctx.enter_context(tc.tile_pool(name="ps_mm", bufs=3, space="PSUM"))
        ps_sm = ctx.enter_context(tc.tile_pool(name="ps_sm", bufs=2, space="PSUM"))

        # ---------------- constants / weights ----------------
        def cdma(name, shape, dt, src, eng=None):
            t = consts.tile(shape, dt, name=name, tag=name)
            (eng or nc.sync).dma_start(out=t, in_=src)
            return t

        # residual stream: ONE [P, KT, S] tile (k-tiles as column blocks) so
        # the per-chunk h0 fill is a single strided DMA; filled per-chunk
        # inside the layer-1 loop so the a1/v1 streams win the DMA queue
        hbig = hpool.tile([P, KT, S], MM, name="hbig", tag="hbig")

        def hsl(k, s0, s1):
            return hbig[:, k, s0:s1]

        eps_sb = consts.tile([P, 1], F32, name="eps", tag="eps")
        nc.vector.memset(eps_sb, EPS_RMS)  # EPS_RMS == EPS_LN

        # chunk-0 a/v per-k tiles: separate tiles so each k's scan starts as
        # soon as ITS two DMAs land (tile-granular deps would otherwise make
        # the first scan wait for every chunk-0 descriptor). These DMAs are
        # emitted FIRST so they win the sync ring over the weight consts.
        a0t, v0t = [], []
        for k in range(KT):
            at0 = work.tile([P, SC], SCD, name=f"a0_{k}", tag="c0av", bufs=8)
            vt0 = work.tile([P, SC], SCD, name=f"v0_{k}", tag="c0av", bufs=8)
            nc.sync.dma_start(out=at0, in_=d_a1T[0][:, k * SC:(k + 1) * SC])
            nc.sync.dma_start(out=vt0, in_=d_v1T[0][:, k * SC:(k + 1) * SC])
            a0t.append(at0)
            v0t.append(vt0)
        nc.scalar.dma_start(out=hbig[:, :, 0:SC], in_=d_h0T[0])

        # constants needed by layer 1 (behind the chunk-0 streams)
        ident_sb = cdma("ident", [P, P], MM, d_ident[:, :])
        bout_sb = {}
        for k in range(KT):
            bout_sb[0, k] = cdma(f"bout_0_{k}", [P, 1], F32, d_bout[0, k])
        wot = {}
        for k in range(KT):
            wot[0, k] = cdma(f"wot_0_{k}", [P, D], MM, d_WoT[0, k])

        def load_early_consts():
            # tiny ones-vectors: needed by layer-2 stats that overlap L1
            nonlocal onesk_sb, onesr_sb
            onesk_sb = cdma("onesk", [P, 1], MM, d_onesk[:, :])
            onesr_sb = cdma("onesr", [1, P], MM, d_onesr[:, :])

        def load_late_consts():
            # layer-2 weights ride the sync ring (1.5MB, needed ~15us after
            # this point); everything later goes on the scalar ring behind
            # the h0 streams, ordered by first use so nothing arrives late
            nonlocal cwrep_sb, onessel_sb, b2_sb, w1, w2sb, identf_sb
            for k in range(KT):
                w1[1, k] = cdma(f"w1_1_{k}", [P, 2 * D], MM, d_W1[0, k])
                wot[1, k] = cdma(f"wot_1_{k}", [P, D], MM, d_WoT[1, k])
                bout_sb[1, k] = cdma(f"bout_1_{k}", [P, 1], F32, d_bout[1, k])
            for li in range(2, L):
                for k in range(KT):
                    w1[li, k] = cdma(f"w1_{li}_{k}", [P, 2 * D], MM,
                                     d_W1[li - 1, k], nc.scalar)
                    wot[li, k] = cdma(f"wot_{li}_{k}", [P, D], MM,
                                      d_WoT[li, k], nc.scalar)
                    bout_sb[li, k] = cdma(f"bout_{li}_{k}", [P, 1], F32,
                                          d_bout[li, k], nc.scalar)
            w2sb = [cdma(f"w2_{k}", [P, V], MM, d_W2[k], nc.scalar)
                    for k in range(KT)]
            b2_sb = cdma("b2", [P, V], F32, d_b2rep[:, :], nc.scalar)
            cwrep_sb = cdma("cwrep", [P, V], F32, d_cwrep[:, :], nc.scalar)
            onessel_sb = cdma("onessel", [P, 16], MM, d_onessel[:, :],
                              nc.scalar)
            identf_sb = cdma("identf", [4, 4], F32, d_identf[:, :], nc.scalar)

        onesk_sb = onesr_sb = cwrep_sb = onessel_sb = b2_sb = w2sb = None
        identf_sb = None
        w1 = {}

        # ---------------- helpers ----------------
        # NOTE on emission order: the Tile scheduler pops data-ready
        # instructions per engine in emission order and COMMITS them to the
        # engine FIFO even when their tile-slot is still occupied. So a tile
        # allocation must never depend (through slot reuse) on same-engine
        # work emitted later. Everything below is therefore emitted fully
        # chunk-interleaved: produce (a, v) for chunk c, scan chunk c, run
        # the output projection for chunk c, then move to chunk c+1.
        prev_scan = [None] * KT   # (tile, width)

        def scan_piece(k, at, vt, width=SC, tag="hgru", bufs=8):
            hg_t = work.tile([P, width], MM, name="hgru", tag=tag, bufs=bufs)
            if prev_scan[k] is None:
                init = 0.0
            else:
                pt, pw = prev_scan[k]
                init = pt[:, pw - 1:pw]
            nc.vector.tensor_tensor_scan(
                hg_t, at, vt, init, op0=AL.mult, op1=AL.add)
            prev_scan[k] = (hg_t, width)
            return hg_t

        def out_proj_chunk(li, c, hg_sl):
            """h[:, chunk c] += W_out.T @ hgru + b_out (resid via identity MM,
            bias via the ACT eviction).  hg_sl(k, sub) -> [P, 512] AP."""
            for m in range(KT):
                ps = ps_mm.tile([P, SC], F32, name="pso", tag="psmm")
                for sub in range(SC // 512):
                    psl = ps[:, sub * 512:(sub + 1) * 512]
                    base = c * SC + sub * 512
                    for k in range(KT):
                        nc.tensor.matmul(
                            psl, wot[li, k][:, m * P:(m + 1) * P],
                            hg_sl(k, sub), start=(k == 0), stop=False)
                    nc.tensor.matmul(psl, ident_sb, hsl(m, base, base + 512),
                                     start=False, stop=True)
                nc.scalar.activation(hsl(m, c * SC, (c + 1) * SC), ps,
                                     AF.Identity, bias=bout_sb[li, m])

        def mk_sl(hgru_c):
            return lambda k, sub: hgru_c[k][:, sub * 512:(sub + 1) * 512]

        def rsqrt_rows(dst, src, p0, p1, scale, eps, tagp):
            """dst[p0:p1] = 1/sqrt(src[p0:p1]*scale + eps) via DVE Newton
            (seed + 2 NR iters) -- keeps the ACT sigmoid table resident."""
            W = src.shape[-1]
            fm = small.tile([P, W], F32, name="fm" + tagp, tag="fm" + tagp,
                            bufs=2)
            fa = small.tile([P, W], F32, name="fa" + tagp, tag="fa" + tagp,
                            bufs=2)
            fb = small.tile([P, W], F32, name="fb" + tagp, tag="fb" + tagp,
                            bufs=2)
            fc = small.tile([P, W], F32, name="fc" + tagp, tag="fc" + tagp,
                            bufs=2)
            sl = (slice(p0, p1), slice(None))
            nc.vector.tensor_scalar(fm[sl], src[sl], scale, eps,
                                    op0=AL.mult, op1=AL.add)
            nc.vector.tensor_scalar(fa.bitcast(I32)[sl], fm.bitcast(I32)[sl],
                                    1, None, op0=AL.logical_shift_right)
            nc.vector.tensor_scalar(fb.bitcast(I32)[sl], fa.bitcast(I32)[sl],
                                    -1, 0x5F3759DF, op0=AL.mult, op1=AL.add)
            for _ in range(2):
                nc.vector.tensor_mul(fa[sl], fb[sl], fb[sl])
                nc.vector.tensor_mul(fc[sl], fa[sl], fm[sl])
                nc.vector.tensor_scalar(fa[sl], fc[sl], -0.5, 1.5,
                                        op0=AL.mult, op1=AL.add)
                nc.vector.tensor_mul(fb[sl], fb[sl], fa[sl])
            nc.vector.tensor_copy(dst[sl], fb[sl])

        # -------- rms stats machinery (pipelined across layers) --------
        def make_rms_stats(li):
            st = {}
            st["rt_raw"] = small.tile([P, S // P], F32, name="rtraw",
                                      tag="rtraw", bufs=2)
            st["rrmm"] = small.tile([P, S // P], MM, name="rrmm", tag="rrmm",
                                    bufs=2)
            st["r_row"] = small.tile([1, S], MM, name="rrow", tag="rrow",
                                     bufs=1)
            st["rbig"] = work.tile([P, S], MM, name="rbig", tag="rbig",
                                   bufs=1)

            def stats_chunk(c):
                sqt = {}
                for k in range(KT):
                    sq = work.tile([P, SC], MM, name="sq", tag="sq", bufs=6)
                    nc.gpsimd.tensor_mul(sq, hsl(k, c * SC, (c + 1) * SC),
                                         hsl(k, c * SC, (c + 1) * SC))
                    sqt[k] = sq
                for sub in range(SC // 512):
                    n = c * (SC // 512) + sub
                    ps = ps_sm.tile([1, 512], F32, name="pstat", tag="small")
                    for k in range(KT):
                        nc.tensor.matmul(
                            ps, onesk_sb, sqt[k][:, sub * 512:(sub + 1) * 512],
                            start=(k == 0), stop=(k == KT - 1))
                    stg = small.tile([1, 512], F32, name="stg", tag="stage",
                                     bufs=3)
                    nc.scalar.copy(out=stg, in_=ps)
                    nc.scalar.dma_start(
                        out=st["rt_raw"][16 * n:16 * (n + 1), :], in_=stg)

            def r_group(p0, p1, s0, s1):
                # r = 1/sqrt(sumsq/D + eps) for stat rows [p0,p1) = tokens
                # [s0,s1), then row-ify and broadcast across partitions
                rsqrt_rows(st["rrmm"], st["rt_raw"], p0, p1, 1.0 / D, EPS_RMS,
                           "r")
                nc.scalar.dma_start(out=st["r_row"][:, s0:s1],
                                    in_=st["rrmm"][p0:p1, :])
                for n in range(s0 // 512, s1 // 512):
                    psb = ps_sm.tile([P, 512], F32, name="psb", tag="small")
                    nc.tensor.matmul(psb, onesr_sb,
                                     st["r_row"][:, n * 512:(n + 1) * 512],
                                     start=True, stop=True)
                    nc.scalar.copy(out=st["rbig"][:, n * 512:(n + 1) * 512],
                                   in_=psb)

            st["stats_chunk"] = stats_chunk
            st["r_group"] = r_group
            return st

        # -------- final-LN stats machinery (two halves) --------
        ln = {"mn": None, "nrstd": None}

        def ln_stats_half(half):
            sqf = {}
            for c in range(half * NSC // 2, (half + 1) * NSC // 2):
                for k in range(KT):
                    sq = work.tile([P, SC], MM, name="sqf", tag="sq", bufs=6)
                    nc.gpsimd.tensor_mul(sq, hsl(k, c * SC, (c + 1) * SC),
                                         hsl(k, c * SC, (c + 1) * SC))
                    sqf[k, c] = sq
            # Per-token stats: ones-matmuls write 4 sequence-subs into the 4
            # partition-rows of ONE PSUM [4, 512] tile (one-hot column
            # weights, one long accumulation chain), evicted once and flipped
            # to token-on-partition [P, 16] via PE transposes. Column layout
            # j-major: col = j*4 + n_local.
            psg_m = ps_sm.tile([4, 512], F32, name="psgm", tag="small")
            psg_q = ps_sm.tile([4, 512], F32, name="psgq", tag="small")
            nmm = 0
            for nl, n in enumerate(range(half * 4, half * 4 + 4)):
                for k in range(KT):
                    nc.tensor.matmul(psg_m, onessel_sb[:, nl * 4:(nl + 1) * 4],
                                     hsl(k, n * 512, (n + 1) * 512),
                                     start=(nmm == 0), stop=(nmm == 15))
                    nmm += 1
            nmm = 0
            for nl, n in enumerate(range(half * 4, half * 4 + 4)):
                c, sub = divmod(n, SC // 512)
                for k in range(KT):
                    nc.tensor.matmul(psg_q, onessel_sb[:, nl * 4:(nl + 1) * 4],
                                     sqf[k, c][:, sub * 512:(sub + 1) * 512],
                                     start=(nmm == 0), stop=(nmm == 15))
                    nmm += 1
            stage_m = small.tile([4, 512], F32, name="stgm", tag="stgm",
                                 bufs=2)
            stage_q = small.tile([4, 512], F32, name="stgq", tag="stgq",
                                 bufs=2)
            nc.scalar.copy(out=stage_m, in_=psg_m)
            nc.scalar.copy(out=stage_q, in_=psg_q)
            HC = 16                        # stat columns per half
            mth = small.tile([P, HC], F32, name="mth", tag="mth", bufs=2)
            qth = small.tile([P, HC], F32, name="qth", tag="qth", bufs=2)
            for j in range(4):
                pst = ps_sm.tile([P, 4], F32, name="pst", tag="small")
                nc.tensor.transpose(pst, stage_m[0:4, j * P:(j + 1) * P],
                                    identf_sb[:, :])
                nc.scalar.copy(out=mth[:, j * 4:(j + 1) * 4], in_=pst)
                psu = ps_sm.tile([P, 4], F32, name="psu", tag="small")
                nc.tensor.transpose(psu, stage_q[0:4, j * P:(j + 1) * P],
                                    identf_sb[:, :])
                nc.scalar.copy(out=qth[:, j * 4:(j + 1) * 4], in_=psu)
            # rstd = 1/sqrt(E[h^2] - E[h]^2 + eps) for this half's columns
            mn = small.tile([P, HC], F32, name="mn", tag="mn", bufs=2)
            nc.vector.tensor_scalar_mul(mn, mth, 1.0 / D)
            qn = small.tile([P, HC], F32, name="qn", tag="qn", bufs=2)
            nc.vector.tensor_scalar_mul(qn, qth, 1.0 / D)
            msq = small.tile([P, HC], F32, name="msq", tag="msq", bufs=2)
            nc.vector.tensor_mul(msq, mn, mn)
            var = small.tile([P, HC], F32, name="var", tag="var", bufs=2)
            nc.vector.tensor_sub(var, qn, msq)
            rstd = small.tile([P, HC], F32, name="rstd", tag="rstd", bufs=2)
            rsqrt_rows(rstd, var, 0, P, 1.0, EPS_LN, "l")
            nrstd = small.tile([P, HC], F32, name="nrstd", tag="nrstd",
                               bufs=2)
            nc.vector.tensor_scalar_mul(nrstd, rstd, -1.0)
            ln["mn"], ln["nrstd"] = mn, nrstd

        def ln_logits_half(half):
            # logits[s,v] = rstd[s]*((W2.T h)[s,v] - mu[s]*cw[v]) + b2[v]
            #             = (mu*cw - psl) * (-rstd) + b2
            # PSUM evicted to SBUF on ACT; corrections on GpSimd (frees DVE)
            mn, nrstd = ln["mn"], ln["nrstd"]
            ostage = None
            SH = S // 2
            for sti in range(SH // P):
                st = half * (SH // P) + sti
                psl = ps_sm.tile([P, V], F32, name="psl", tag="small")
                for k in range(KT):
                    nc.tensor.matmul(psl, hsl(k, st * P, (st + 1) * P),
                                     w2sb[k], start=(k == 0),
                                     stop=(k == KT - 1))
                g, j = divmod(st, 4)
                rcol = (sti % 4) * 4 + sti // 4
                if j == 0:
                    ostage = small.tile([P, 4 * V], F32, name="ostage",
                                        tag="ostage", bufs=2)
                t1 = work.tile([P, V], F32, name="t1", tag="t1", bufs=2)
                nc.vector.scalar_tensor_tensor(
                    t1, cwrep_sb, mn[:, rcol:rcol + 1], psl,
                    op0=AL.mult, op1=AL.subtract)
                nc.vector.scalar_tensor_tensor(
                    ostage[:, j * V:(j + 1) * V], t1, nrstd[:, rcol:rcol + 1],
                    b2_sb, op0=AL.mult, op1=AL.add)
                if j == 3:
                    nc.sync.dma_start(out=d_out[g], in_=ostage)

        carry = [None]   # (li, hg_sl) of the previous layer's last chunk

        def mid_layer(li, stats, stats_next):
            """rmsnorm -> hidden/gate -> (a,v) -> scan -> out-proj, chunked.
            `stats` (this layer's rms stats) already has chunks 0-1 emitted;
            `stats_next` is the NEXT layer's ctx to pipeline into this loop
            (None for the last layer: final-LN stats are emitted instead)."""
            if carry[0] is not None:
                # previous layer's last out-proj first: fills the PE while
                # this layer's stats chain gets going
                pli, psl_ = carry[0]
                out_proj_chunk(pli, NSC - 1, psl_)
                carry[0] = None
            # stats chunks 0-2 were pipelined into the previous layer's
            # loop; r for tokens of chunks 0-2 is ready immediately.  Chunk
            # 3's stats (which need the carry out-proj's eviction) are
            # emitted inside the c==0 iteration so their PE/gpsimd work
            # never head-of-line-blocks the first hg matmuls.
            stats["r_group"](0, 96, 0, 3 * SC)

            rbig = stats["rbig"]

            def make_rmst(c):
                t = {}
                for k in range(KT):
                    rm = work.tile([P, SC], MM, name="rms", tag="rms", bufs=8)
                    nc.vector.tensor_mul(rm, hsl(k, c * SC, (c + 1) * SC),
                                         rbig[:, c * SC:(c + 1) * SC])
                    t[k] = rm
                return t

            pending = {}
            rmst = make_rmst(0)
            rmst_next = None
            for c in range(NSC):
                ght = {}
                hgru_c = {}
                for m in [x for k in range(KT) for x in (k, k + KT)]:
                    ps = ps_mm.tile([P, SC], F32, name="psh", tag="psmm")
                    for sub in range(SC // 512):
                        psl = ps[:, sub * 512:(sub + 1) * 512]
                        for k in range(KT):
                            nc.tensor.matmul(
                                psl, w1[li, k][:, m * P:(m + 1) * P],
                                rmst[k][:, sub * 512:(sub + 1) * 512],
                                start=(k == 0), stop=(k == KT - 1))
                    if m < KT:   # hidden half: g = max(hid + 0.5, sigmoid(hid))
                        sg = work.tile([P, SC], SCD, name="sg", tag="sg",
                                       bufs=2)
                        nc.scalar.activation(sg, ps, AF.Sigmoid)
                        gh = work.tile([P, SC], SCD, name="gh", tag="gh",
                                       bufs=2)
                        nc.vector.scalar_tensor_tensor(
                            gh, ps, 0.5, sg, op0=AL.add, op1=AL.max)
                        ght[m] = gh
                        if m == KT - 1 and c < NSC - 1:
                            # prefetch next chunk's rmsnormed tiles on the
                            # DVE ahead of this chunk's last scans, so the
                            # PE's hg(c+1) matmuls never wait on the DVE
                            # draining chunk c
                            rmst_next = make_rmst(c + 1)
                    else:        # gate half: a = sigmoid(-gate); vneg = (a-1)*g
                        k = m - KT
                        at = work.tile([P, SC], SCD, name="a_t", tag="a_t",
                                       bufs=2)
                        nc.scalar.activation(at, ps, AF.Sigmoid, scale=-1.0)
                        vt = work.tile([P, SC], SCD, name="v_t", tag="v_t",
                                       bufs=2)
                        nc.vector.scalar_tensor_tensor(
                            vt, at, 1.0, ght[k], op0=AL.subtract, op1=AL.mult)
                        hgru_c[k] = scan_piece(k, at, vt)
                pending[c] = mk_sl(hgru_c)
                if c == 0:
                    stats["stats_chunk"](3)
                    stats["r_group"](96, 128, 3 * SC, S)
                if c >= 1:
                    out_proj_chunk(li, c - 1, pending.pop(c - 1))
                    if c >= 2:
                        # pipeline the NEXT layer's stats for chunk c-2 (its
                        # h was finalized one chunk ago -- fully data-ready,
                        # so these PE ones-matmuls never stall the queue)
                        if stats_next is not None:
                            stats_next["stats_chunk"](c - 2)
                            if c == NSC - 1:
                                stats_next["stats_chunk"](2)
                        elif c == NSC - 1:
                            # last layer: final-LN stats for half 0
                            ln_stats_half(0)
                rmst = rmst_next
            carry[0] = (li, pending.pop(NSC - 1))

        # ---------------- layer 1 (token-LUT scan inputs) ----------------
        # a/v stream per-k [P, SC] tiles on the sync HWDGE ring (each k's
        # scan starts as soon as its own DMA lands), h0 on the scalar ring
        # (the two rings run in parallel)
        stats2 = None
        l1_pending = {}
        for c in range(NSC):
            if c == 0:
                # chunk 0: tiles already DMA'd (emitted first, above);
                # scan in 512-halves into separate tiles so the first
                # out-proj sub can start after only the first-half scans
                halves = {}
                for h in range(2):
                    for k in range(KT):
                        halves[k, h] = scan_piece(
                            k, a0t[k][:, h * 512:(h + 1) * 512],
                            v0t[k][:, h * 512:(h + 1) * 512],
                            width=512, tag="hgruh", bufs=8)
                l1_pending[0] = lambda k, sub: halves[k, sub][:, :]
                load_early_consts()
                continue
            nc.scalar.dma_start(out=hbig[:, :, c * SC:(c + 1) * SC],
                                in_=d_h0T[c])
            hgru_c = {}
            for k in range(KT):
                apk = work.tile([P, SC], SCD, name="a_pk", tag="c0av",
                                bufs=8)
                vpk = work.tile([P, SC], SCD, name="v_pk", tag="c0av",
                                bufs=8)
                nc.sync.dma_start(out=apk,
                                  in_=d_a1T[c][:, k * SC:(k + 1) * SC])
                nc.sync.dma_start(out=vpk,
                                  in_=d_v1T[c][:, k * SC:(k + 1) * SC])
                hgru_c[k] = scan_piece(k, apk, vpk)
            l1_pending[c] = mk_sl(hgru_c)
            out_proj_chunk(0, c - 1, l1_pending.pop(c - 1))
            if c == 1:
                load_late_consts()
                stats2 = make_rms_stats(1)
            if c >= 2:
                stats2["stats_chunk"](c - 2)
            if c == NSC - 1:
                stats2["stats_chunk"](2)
        carry[0] = (0, l1_pending.pop(NSC - 1))

        stats = stats2
        for li in range(1, L):
            prev_scan = [None] * KT
            stats_next = make_rms_stats(li + 1) if li < L - 1 else None
            mid_layer(li, stats, stats_next)
            stats = stats_next
        pli, phg = carry[0]
        out_proj_chunk(pli, NSC - 1, phg)
        carry[0] = None

        # ---------------- final layernorm + vocab projection ----------------
        # half-0 stats were pipelined into layer L-1's loop; emit its logits
        # now, then half-1 stats + logits. Fences order each half's LN-stat
        # PSUM work before its logits matmuls (shared small PSUM tag).
        tc.no_sync_barrier()
        ln_logits_half(0)
        ln_stats_half(1)
        tc.no_sync_barrier()
        ln_logits_half(1)

    nc.finalize()
    return nc


def _sigmoid(x):
    return 1.0 / (1.0 + np.exp(-x))


def _host_prep(inputs):
    """Precompute per-core device inputs (numpy, float64 internal)."""
    mm_np = _np_dt(MM_KIND)
    sc_np = _np_dt(SC_KIND)

    x = np.asarray(inputs["x"]).astype(np.int64)        # [B, S]
    emb = np.asarray(inputs["emb"]).astype(np.float64)  # [V, D]
    rms_w = np.asarray(inputs["rms_w"]).astype(np.float64)
    W_hg = np.asarray(inputs["W_hg"]).astype(np.float64)
    W_out = np.asarray(inputs["W_out"]).astype(np.float64)
    b_out = np.asarray(inputs["b_out"]).astype(np.float64)
    ln_w = np.asarray(inputs["ln_w"]).astype(np.float64)
    ln_b = np.asarray(inputs["ln_b"]).astype(np.float64)
    out_W = np.asarray(inputs["out_W"]).astype(np.float64)
    out_b = np.asarray(inputs["out_b"]).astype(np.float64)

    # layer-1 token tables
    r0 = 1.0 / np.sqrt((emb * emb).mean(-1, keepdims=True) + EPS_RMS)
    rms1 = emb * r0 * rms_w[0][None, :]
    hg1 = rms1 @ W_hg[0].T
    hid1, gate1 = hg1[:, :D], hg1[:, D:]
    z1 = _sigmoid(gate1)
    a_tab = _sigmoid(-gate1)                               # [V, D]
    # device scans -v and the sign is folded into W_out
    v_tab = -(z1 * np.maximum(hid1 + 0.5, _sigmoid(hid1)))  # [V, D]

    # folded weights (W_out negated: the scan output is -h_gru)
    W1 = np.stack([(W_hg[i] * rms_w[i][None, :]).T for i in range(1, L)])
    W1 = W1.reshape(L - 1, KT, P, 2 * D)
    WoT = np.stack([-W_out[i].T for i in range(L)]).reshape(L, KT, P, D)
    W2 = (out_W * ln_w[None, :]).T                         # [D, V]
    cw = W2.sum(0)[None, :]                                # [1, V]
    b2 = out_b + out_W @ ln_b                              # [V]

    common = {
        "W1": W1.astype(mm_np),
        "WoT": WoT.astype(mm_np),
        "W2": W2.reshape(KT, P, V).astype(mm_np),
        "bout": np.ascontiguousarray(
            b_out.reshape(L, KT, P, 1)).astype(np.float32),
        "b2rep": np.tile(b2[None, :].astype(np.float32), (P, 1)),
        "cwrep": np.tile(cw.astype(np.float32), (P, 1)),
        "onessel": np.tile(np.eye(4).reshape(-1), (P, 1)).astype(mm_np),
        "onesk": np.ones((P, 1), mm_np),
        "onesr": np.ones((1, P), mm_np),
        "ident": np.eye(P, dtype=np.float32).astype(mm_np),
        "identf": np.eye(4, dtype=np.float32),
    }
    def pack(tab_rows):
        # [S, D] token-gathered rows -> [NSC, P, KT*SC] chunk-major pack
        t = tab_rows.T.reshape(KT, P, NSC, SC)        # [k, p, c, j]
        return np.ascontiguousarray(t.transpose(2, 1, 0, 3)
                                    .reshape(NSC, P, KT * SC))

    in_maps = []
    for b in range(B):
        tok = x[b]
        m = dict(common)
        m["h0T"] = pack(emb[tok]).astype(mm_np)
        m["a1T"] = pack(a_tab[tok]).astype(sc_np)
        m["v1T"] = pack(v_tab[tok]).astype(sc_np)
        in_maps.append(m)
    return in_maps


def _get_nc():
    if "nc" not in _cache:
        _cache["nc"] = _build_nc()
    return _cache["nc"]


def kernel(**inputs):
    from concourse.bass_utils import run_bass_kernel_spmd

    nc = _get_nc()
    in_maps = _host_prep(inputs)
    res = run_bass_kernel_spmd(nc, in_maps, core_ids=list(range(N_CORES)),
                               trace=bool(int(os.environ.get("EM_TRACE", "0"))))
    _cache["last_result"] = res
    # device layout [g, p, j*V+v] -> token s = g*1024 + j*128 + p
    out = np.stack([
        np.asarray(r["out"]).reshape(S // (4 * P), P, 4, V)
        .transpose(0, 2, 1, 3).reshape(S, V)
        for r in res.results
    ], axis=0)
    return out.astype(np.float32)


# revision 16
# speedup vs baseline: 1.0514x; 1.0514x over previous
"""Trainium2 Bass kernel for nn_EntropyModel (minGRU LM).

Strategy (8 NeuronCores, data-parallel over batch B=8, one sample per core):

  - Residual stream kept TRANSPOSED on device: hT[d, s] (d on partitions,
    s along free dim), because the minGRU recurrence is computed with the
    DVE `tensor_tensor_scan` instruction (state = a*state + v along the
    free dim, fp32 internal state) which needs lanes on partitions and
    time along free.
  - Layer 1 is token-lookup: rms/hg/nonlinearities of layer 1 depend only
    on the token id (vocab=256), so a1/v1 (scan coefficients/values) and
    h0 (embedding) are precomputed on host as tables and gathered per
    token; the device only runs the scan + out-projection for layer 1.
  - minGRU math:  a = sigmoid(-gate) = 1 - z,  v = z * g(hidden) with
    g(x) = max(x + 0.5, sigmoid(x))  (exact identity for the reference's
    where(x>=0, x+0.5, sigmoid(x))).
    Device computes a directly via sigmoid(-gate) (ACT scale=-1) and
    vneg = (a - 1) * g = -v in ONE scalar_tensor_tensor op; the scan then
    produces -h_gru and the sign is folded into W_out host-side.
  - rmsnorm weight folded into W_hg host-side; layernorm w/b folded into
    the vocab projection: logits = rstd*(W2.T h - mu*colsum(W2)) + b2.
  - b_out rides the out-projection PSUM eviction as the ACT bias operand
    (out = Copy(ps + b_out)); the residual add stays an identity matmul
    inside the PSUM accumulation.
  - Per-token sums over d (rms sumsq, LN mean/meansq) via ones-vector
    matmuls on the PE; per-free broadcast of r[s] via K=1 rank-1 matmuls.
  - Engine balance: squares for the stats run on the (otherwise idle)
    GpSimd engine as fp8 tiles; the final logits corrections also run on
    GpSimd after an ACT PSUM eviction; all rsqrts use the DVE Newton
    iteration so the ACT sigmoid table stays resident (ACT table reloads
    cost 1.3us each).
  - Stats for layer li+1 are emitted inside layer li's chunk loop (chunk
    c-2 after out-proj of c-1) so layer boundaries don't stall the PE;
    the final-LN stats for the first half ride inside layer L-1's loop.

Matmul/bf16 dtype knobs below (MM_*: matmul operands, SC_*: scan operands).
"""

import os
import numpy as np
import ml_dtypes

V, D, L, B, S = 256, 512, 4, 8, 4096
EPS_RMS = 1e-5
EPS_LN = 1e-5
P = 128
KT = D // P            # 4 d-tiles of 128
ET = 2 * D // P        # 8 e-tiles for the hidden/gate projection
SC = 1024              # s-chunk for working tiles
NSC = S // SC
N_CORES = 8

# dtype knobs: "bf16" or "f32r" for matmul operands, "bf16"/"f32" for scan
MM_KIND = os.environ.get("EM_MM_KIND", "bf16")
SC_KIND = os.environ.get("EM_SC_KIND", "bf16")

_cache = {}


def _np_dt(kind):
    return np.float32 if kind in ("f32", "f32r") else ml_dtypes.bfloat16


def _build_nc():
    import concourse.bass as bass  # noqa: F401
    import concourse.bacc as bacc
    import concourse.mybir as mybir
    import concourse.tile as tile
    from contextlib import ExitStack

    AL = mybir.AluOpType
    AF = mybir.ActivationFunctionType
    F32 = mybir.dt.float32
    I32 = mybir.dt.int32
    F8 = mybir.dt.float8e4
    MM = {"bf16": mybir.dt.bfloat16, "f32r": mybir.dt.float32r}[MM_KIND]
    SCD = {"bf16": mybir.dt.bfloat16, "f32": mybir.dt.float32}[SC_KIND]

    nc = bacc.Bacc()

    # layer-1 streams packed chunk-major: [c, p, k*SC+j] = orig[k*128+p, c*SC+j]
    # so each chunk is ONE ~1MB DMA (descriptor overhead amortized)
    d_h0T = nc.dram_tensor("h0T", [NSC, P, KT * SC], MM, kind="ExternalInput")
    d_a1T = nc.dram_tensor("a1T", [NSC, P, KT * SC], SCD, kind="ExternalInput")
    d_v1T = nc.dram_tensor("v1T", [NSC, P, KT * SC], SCD, kind="ExternalInput")
    d_W1 = nc.dram_tensor("W1", [L - 1, KT, P, 2 * D], MM, kind="ExternalInput")
    d_WoT = nc.dram_tensor("WoT", [L, KT, P, D], MM, kind="ExternalInput")
    d_W2 = nc.dram_tensor("W2", [KT, P, V], MM, kind="ExternalInput")
    d_bout = nc.dram_tensor("bout", [L, KT, P, 1], F32, kind="ExternalInput")
    d_b2rep = nc.dram_tensor("b2rep", [P, V], F32, kind="ExternalInput")
    d_cwrep = nc.dram_tensor("cwrep", [P, V], F32, kind="ExternalInput")
    d_onessel = nc.dram_tensor("onessel", [P, 16], MM, kind="ExternalInput")
    d_onesk = nc.dram_tensor("onesk", [P, 1], MM, kind="ExternalInput")
    d_onesr = nc.dram_tensor("onesr", [1, P], MM, kind="ExternalInput")
    d_ident = nc.dram_tensor("ident", [P, P], MM, kind="ExternalInput")
    d_identf = nc.dram_tensor("identf", [4, 4], F32, kind="ExternalInput")
    # output laid out [group, partition, 4*V]: token s = g*512 + j*128 + p
    # lives at out[g, p, j*V:(j+1)*V] so a group of 4 token-tiles is one
    # contiguous [128, 1024] DMA; the host un-permutes.
    d_out = nc.dram_tensor("out", [S // (4 * P), P, 4 * V], F32,
                           kind="ExternalOutput")

    with ExitStack() as ctx:
        tc = ctx.enter_context(tile.TileContext(nc))
        consts = ctx.enter_context(tc.tile_pool(name="consts", bufs=1))
        hpool = ctx.enter_context(tc.tile_pool(name="hpool", bufs=1))
        work = ctx.enter_context(tc.tile_pool(name="work", bufs=2))
        small = ctx.enter_context(tc.tile_pool(name="small", bufs=1))
        ps_mm = ctx.enter_context(tc.tile_pool(name="ps_mm", bufs=3, space="PSUM"))
        ps_sm = ctx.enter_context(tc.tile_pool(name="ps_sm", bufs=2, space="PSUM"))

        # ---------------- constants / weights ----------------
        def cdma(name, shape, dt, src, eng=None):
            t = consts.tile(shape, dt, name=name, tag=name)
            (eng or nc.sync).dma_start(out=t, in_=src)
            return t

        # residual stream: ONE [P, KT, S] tile (k-tiles as column blocks) so
        # the per-chunk h0 fill is a single strided DMA; filled per-chunk
        # inside the layer-1 loop so the a1/v1 streams win the DMA queue
        hbig = hpool.tile([P, KT, S], MM, name="hbig", tag="hbig")

        def hsl(k, s0, s1):
            return hbig[:, k, s0:s1]

        eps_sb = consts.tile([P, 1], F32, name="eps", tag="eps")
        nc.vector.memset(eps_sb, EPS_RMS)  # EPS_RMS == EPS_LN

        # chunk-0 a/v per-k tiles: separate tiles AND scan emission
        # interleaved right behind each k's two DMAs.  DMA-completion
        # semaphores are cumulative per queue: a consumer waits for every
        # DMA emitted on that ring before it, so the scan for k must be
        # emitted before k+1's DMAs to start early.  (Scans are emitted in
        # the layer-1 loop below; here we only stage the tiles + DMAs
        # handles so the loop can interleave.)
        a0t, v0t = [], []

        def c0_dma(k):
            at0 = work.tile([P, SC], SCD, name=f"a0_{k}", tag="c0av", bufs=8)
            vt0 = work.tile([P, SC], SCD, name=f"v0_{k}", tag="c0av", bufs=8)
            nc.sync.dma_start(out=at0, in_=d_a1T[0][:, k * SC:(k + 1) * SC])
            nc.sync.dma_start(out=vt0, in_=d_v1T[0][:, k * SC:(k + 1) * SC])
            a0t.append(at0)
            v0t.append(vt0)

        # layer-1 consts ride the scalar ring behind the h0 stream so they
        # never delay the sync-ring a/v scans (needed only by the first
        # out-proj at ~14us)
        nc.scalar.dma_start(out=hbig[:, :, 0:SC], in_=d_h0T[0])
        ident_sb = cdma("ident", [P, P], MM, d_ident[:, :], nc.scalar)
        bout_sb = {}
        for k in range(KT):
            bout_sb[0, k] = cdma(f"bout_0_{k}", [P, 1], F32, d_bout[0, k],
                                 nc.scalar)
        wot = {}
        for k in range(KT):
            wot[0, k] = cdma(f"wot_0_{k}", [P, D], MM, d_WoT[0, k],
                             nc.scalar)

        def load_early_consts():
            # tiny ones-vectors: needed by layer-2 stats that overlap L1
            nonlocal onesk_sb, onesr_sb
            onesk_sb = cdma("onesk", [P, 1], MM, d_onesk[:, :])
            onesr_sb = cdma("onesr", [1, P], MM, d_onesr[:, :])

        def load_late_consts():
            # layer-2 weights ride the sync ring (1.5MB, needed ~15us after
            # this point); everything later goes on the scalar ring behind
            # the h0 streams, ordered by first use so nothing arrives late
            nonlocal cwrep_sb, onessel_sb, b2_sb, w1, w2sb, identf_sb
            for k in range(KT):
                w1[1, k] = cdma(f"w1_1_{k}", [P, 2 * D], MM, d_W1[0, k])
                wot[1, k] = cdma(f"wot_1_{k}", [P, D], MM, d_WoT[1, k])
                bout_sb[1, k] = cdma(f"bout_1_{k}", [P, 1], F32, d_bout[1, k])
            for li in range(2, L):
                for k in range(KT):
                    w1[li, k] = cdma(f"w1_{li}_{k}", [P, 2 * D], MM,
                                     d_W1[li - 1, k], nc.scalar)
                    wot[li, k] = cdma(f"wot_{li}_{k}", [P, D], MM,
                                      d_WoT[li, k], nc.scalar)
                    bout_sb[li, k] = cdma(f"bout_{li}_{k}", [P, 1], F32,
                                          d_bout[li, k], nc.scalar)
            w2sb = [cdma(f"w2_{k}", [P, V], MM, d_W2[k], nc.scalar)
                    for k in range(KT)]
            b2_sb = cdma("b2", [P, V], F32, d_b2rep[:, :], nc.scalar)
            cwrep_sb = cdma("cwrep", [P, V], F32, d_cwrep[:, :], nc.scalar)
            onessel_sb = cdma("onessel", [P, 16], MM, d_onessel[:, :],
                              nc.scalar)
            identf_sb = cdma("identf", [4, 4], F32, d_identf[:, :], nc.scalar)

        onesk_sb = onesr_sb = cwrep_sb = onessel_sb = b2_sb = w2sb = None
        identf_sb = None
        w1 = {}

        # ---------------- helpers ----------------
        # NOTE on emission order: the Tile scheduler pops data-ready
        # instructions per engine in emission order and COMMITS them to the
        # engine FIFO even when their tile-slot is still occupied. So a tile
        # allocation must never depend (through slot reuse) on same-engine
        # work emitted later. Everything below is therefore emitted fully
        # chunk-interleaved: produce (a, v) for chunk c, scan chunk c, run
        # the output projection for chunk c, then move to chunk c+1.
        prev_scan = [None] * KT   # (tile, width)

        def scan_piece(k, at, vt, width=SC, tag="hgru", bufs=8):
            hg_t = work.tile([P, width], MM, name="hgru", tag=tag, bufs=bufs)
            if prev_scan[k] is None:
                init = 0.0
            else:
                pt, pw = prev_scan[k]
                init = pt[:, pw - 1:pw]
            nc.vector.tensor_tensor_scan(
                hg_t, at, vt, init, op0=AL.mult, op1=AL.add)
            prev_scan[k] = (hg_t, width)
            return hg_t

        def out_proj_chunk(li, c, hg_sl):
            """h[:, chunk c] += W_out.T @ hgru + b_out (resid via identity MM,
            bias via the ACT eviction).  hg_sl(k, sub) -> [P, 512] AP."""
            for m in range(KT):
                ps = ps_mm.tile([P, SC], F32, name="pso", tag="psmm")
                for sub in range(SC // 512):
                    psl = ps[:, sub * 512:(sub + 1) * 512]
                    base = c * SC + sub * 512
                    for k in range(KT):
                        nc.tensor.matmul(
                            psl, wot[li, k][:, m * P:(m + 1) * P],
                            hg_sl(k, sub), start=(k == 0), stop=False)
                    nc.tensor.matmul(psl, ident_sb, hsl(m, base, base + 512),
                                     start=False, stop=True)
                nc.scalar.activation(hsl(m, c * SC, (c + 1) * SC), ps,
                                     AF.Identity, bias=bout_sb[li, m])

        def mk_sl(hgru_c):
            return lambda k, sub: hgru_c[k][:, sub * 512:(sub + 1) * 512]

        def rsqrt_rows(dst, src, p0, p1, scale, eps, tagp):
            """dst[p0:p1] = 1/sqrt(src[p0:p1]*scale + eps) via DVE Newton
            (seed + 2 NR iters) -- keeps the ACT sigmoid table resident."""
            W = src.shape[-1]
            fm = small.tile([P, W], F32, name="fm" + tagp, tag="fm" + tagp,
                            bufs=2)
            fa = small.tile([P, W], F32, name="fa" + tagp, tag="fa" + tagp,
                            bufs=2)
            fb = small.tile([P, W], F32, name="fb" + tagp, tag="fb" + tagp,
                            bufs=2)
            fc = small.tile([P, W], F32, name="fc" + tagp, tag="fc" + tagp,
                            bufs=2)
            sl = (slice(p0, p1), slice(None))
            nc.vector.tensor_scalar(fm[sl], src[sl], scale, eps,
                                    op0=AL.mult, op1=AL.add)
            nc.vector.tensor_scalar(fa.bitcast(I32)[sl], fm.bitcast(I32)[sl],
                                    1, None, op0=AL.logical_shift_right)
            nc.vector.tensor_scalar(fb.bitcast(I32)[sl], fa.bitcast(I32)[sl],
                                    -1, 0x5F3759DF, op0=AL.mult, op1=AL.add)
            for _ in range(2):
                nc.vector.tensor_mul(fa[sl], fb[sl], fb[sl])
                nc.vector.tensor_mul(fc[sl], fa[sl], fm[sl])
                nc.vector.tensor_scalar(fa[sl], fc[sl], -0.5, 1.5,
                                        op0=AL.mult, op1=AL.add)
                nc.vector.tensor_mul(fb[sl], fb[sl], fa[sl])
            nc.vector.tensor_copy(dst[sl], fb[sl])

        # -------- rms stats machinery (pipelined across layers) --------
        def make_rms_stats(li):
            st = {}
            st["rt_raw"] = small.tile([P, S // P], F32, name="rtraw",
                                      tag="rtraw", bufs=2)
            st["rrmm"] = small.tile([P, S // P], MM, name="rrmm", tag="rrmm",
                                    bufs=2)
            st["r_row"] = small.tile([1, S], MM, name="rrow", tag="rrow",
                                     bufs=1)
            st["rbig"] = work.tile([P, S], MM, name="rbig", tag="rbig",
                                   bufs=1)

            def stats_chunk(c):
                # squares split ACT/DVE (gpsimd would contend for the DVE
                # SBUF port and slow every scan/STT by ~15%)
                sqt = {}
                for k in range(KT):
                    sq = work.tile([P, SC], MM, name="sq", tag="sq", bufs=6)
                    if k % 2 == 0:
                        nc.scalar.activation(
                            sq, hsl(k, c * SC, (c + 1) * SC), AF.Square)
                    else:
                        nc.vector.tensor_mul(sq, hsl(k, c * SC, (c + 1) * SC),
                                             hsl(k, c * SC, (c + 1) * SC))
                    sqt[k] = sq
                for sub in range(SC // 512):
                    n = c * (SC // 512) + sub
                    ps = ps_sm.tile([1, 512], F32, name="pstat", tag="small")
                    for k in range(KT):
                        nc.tensor.matmul(
                            ps, onesk_sb, sqt[k][:, sub * 512:(sub + 1) * 512],
                            start=(k == 0), stop=(k == KT - 1))
                    stg = small.tile([1, 512], F32, name="stg", tag="stage",
                                     bufs=3)
                    nc.scalar.copy(out=stg, in_=ps)
                    nc.scalar.dma_start(
                        out=st["rt_raw"][16 * n:16 * (n + 1), :], in_=stg)

            def r_group(p0, p1, s0, s1):
                # r = 1/sqrt(sumsq/D + eps) for stat rows [p0,p1) = tokens
                # [s0,s1), then row-ify and broadcast across partitions
                rsqrt_rows(st["rrmm"], st["rt_raw"], p0, p1, 1.0 / D, EPS_RMS,
                           "r")
                nc.scalar.dma_start(out=st["r_row"][:, s0:s1],
                                    in_=st["rrmm"][p0:p1, :])
                for n in range(s0 // 512, s1 // 512):
                    psb = ps_sm.tile([P, 512], F32, name="psb", tag="small")
                    nc.tensor.matmul(psb, onesr_sb,
                                     st["r_row"][:, n * 512:(n + 1) * 512],
                                     start=True, stop=True)
                    nc.scalar.copy(out=st["rbig"][:, n * 512:(n + 1) * 512],
                                   in_=psb)

            st["stats_chunk"] = stats_chunk
            st["r_group"] = r_group
            return st

        # -------- final-LN stats machinery (two halves) --------
        ln = {"mn": None, "nrstd": None}

        def ln_stats_half(half):
            sqf = {}
            for c in range(half * NSC // 2, (half + 1) * NSC // 2):
                for k in range(KT):
                    sq = work.tile([P, SC], MM, name="sqf", tag="sq", bufs=6)
                    if k % 2 == 0:
                        nc.scalar.activation(
                            sq, hsl(k, c * SC, (c + 1) * SC), AF.Square)
                    else:
                        nc.vector.tensor_mul(sq, hsl(k, c * SC, (c + 1) * SC),
                                             hsl(k, c * SC, (c + 1) * SC))
                    sqf[k, c] = sq
            # Per-token stats: ones-matmuls write 4 sequence-subs into the 4
            # partition-rows of ONE PSUM [4, 512] tile (one-hot column
            # weights, one long accumulation chain), evicted once and flipped
            # to token-on-partition [P, 16] via PE transposes. Column layout
            # j-major: col = j*4 + n_local.
            psg_m = ps_sm.tile([4, 512], F32, name="psgm", tag="small")
            psg_q = ps_sm.tile([4, 512], F32, name="psgq", tag="small")
            nmm = 0
            for nl, n in enumerate(range(half * 4, half * 4 + 4)):
                for k in range(KT):
                    nc.tensor.matmul(psg_m, onessel_sb[:, nl * 4:(nl + 1) * 4],
                                     hsl(k, n * 512, (n + 1) * 512),
                                     start=(nmm == 0), stop=(nmm == 15))
                    nmm += 1
            nmm = 0
            for nl, n in enumerate(range(half * 4, half * 4 + 4)):
                c, sub = divmod(n, SC // 512)
                for k in range(KT):
                    nc.tensor.matmul(psg_q, onessel_sb[:, nl * 4:(nl + 1) * 4],
                                     sqf[k, c][:, sub * 512:(sub + 1) * 512],
                                     start=(nmm == 0), stop=(nmm == 15))
                    nmm += 1
            stage_m = small.tile([4, 512], F32, name="stgm", tag="stgm",
                                 bufs=2)
            stage_q = small.tile([4, 512], F32, name="stgq", tag="stgq",
                                 bufs=2)
            nc.scalar.copy(out=stage_m, in_=psg_m)
            nc.scalar.copy(out=stage_q, in_=psg_q)
            HC = 16                        # stat columns per half
            mth = small.tile([P, HC], F32, name="mth", tag="mth", bufs=2)
            qth = small.tile([P, HC], F32, name="qth", tag="qth", bufs=2)
            for j in range(4):
                pst = ps_sm.tile([P, 4], F32, name="pst", tag="small")
                nc.tensor.transpose(pst, stage_m[0:4, j * P:(j + 1) * P],
                                    identf_sb[:, :])
                nc.scalar.copy(out=mth[:, j * 4:(j + 1) * 4], in_=pst)
                psu = ps_sm.tile([P, 4], F32, name="psu", tag="small")
                nc.tensor.transpose(psu, stage_q[0:4, j * P:(j + 1) * P],
                                    identf_sb[:, :])
                nc.scalar.copy(out=qth[:, j * 4:(j + 1) * 4], in_=psu)
            # rstd = 1/sqrt(E[h^2] - E[h]^2 + eps) for this half's columns
            mn = small.tile([P, HC], F32, name="mn", tag="mn", bufs=2)
            nc.vector.tensor_scalar_mul(mn, mth, 1.0 / D)
            qn = small.tile([P, HC], F32, name="qn", tag="qn", bufs=2)
            nc.vector.tensor_scalar_mul(qn, qth, 1.0 / D)
            msq = small.tile([P, HC], F32, name="msq", tag="msq", bufs=2)
            nc.vector.tensor_mul(msq, mn, mn)
            var = small.tile([P, HC], F32, name="var", tag="var", bufs=2)
            nc.vector.tensor_sub(var, qn, msq)
            rstd = small.tile([P, HC], F32, name="rstd", tag="rstd", bufs=2)
            rsqrt_rows(rstd, var, 0, P, 1.0, EPS_LN, "l")
            nrstd = small.tile([P, HC], F32, name="nrstd", tag="nrstd",
                               bufs=2)
            nc.vector.tensor_scalar_mul(nrstd, rstd, -1.0)
            ln["mn"], ln["nrstd"] = mn, nrstd

        def ln_logits_half(half):
            # logits[s,v] = rstd[s]*((W2.T h)[s,v] - mu[s]*cw[v]) + b2[v]
            #             = (mu*cw - psl) * (-rstd) + b2
            # PSUM evicted to SBUF on ACT; corrections on GpSimd (frees DVE)
            mn, nrstd = ln["mn"], ln["nrstd"]
            ostage = None
            SH = S // 2
            for sti in range(SH // P):
                st = half * (SH // P) + sti
                psl = ps_sm.tile([P, V], F32, name="psl", tag="small")
                for k in range(KT):
                    nc.tensor.matmul(psl, hsl(k, st * P, (st + 1) * P),
                                     w2sb[k], start=(k == 0),
                                     stop=(k == KT - 1))
                g, j = divmod(st, 4)
                rcol = (sti % 4) * 4 + sti // 4
                if j == 0:
                    ostage = small.tile([P, 4 * V], F32, name="ostage",
                                        tag="ostage", bufs=2)
                t1 = work.tile([P, V], F32, name="t1", tag="t1", bufs=2)
                nc.vector.scalar_tensor_tensor(
                    t1, cwrep_sb, mn[:, rcol:rcol + 1], psl,
                    op0=AL.mult, op1=AL.subtract)
                nc.vector.scalar_tensor_tensor(
                    ostage[:, j * V:(j + 1) * V], t1, nrstd[:, rcol:rcol + 1],
                    b2_sb, op0=AL.mult, op1=AL.add)
                if j == 3:
                    nc.sync.dma_start(out=d_out[g], in_=ostage)

        carry = [None]   # (li, hg_sl) of the previous layer's last chunk

        def mid_layer(li, stats, stats_next):
            """rmsnorm -> hidden/gate -> (a,v) -> scan -> out-proj, chunked.
            `stats` (this layer's rms stats) already has chunks 0-1 emitted;
            `stats_next` is the NEXT layer's ctx to pipeline into this loop
            (None for the last layer: final-LN stats are emitted instead)."""
            if carry[0] is not None:
                # previous layer's last out-proj first: fills the PE while
                # this layer's stats chain gets going
                pli, psl_ = carry[0]
                out_proj_chunk(pli, NSC - 1, psl_)
                carry[0] = None
            # stats chunks 0-2 were pipelined into the previous layer's
            # loop; r for tokens of chunks 0-2 is ready immediately.  Chunk
            # 3's stats (which need the carry out-proj's eviction) are
            # emitted inside the c==0 iteration so their PE/gpsimd work
            # never head-of-line-blocks the first hg matmuls.
            stats["r_group"](0, 96, 0, 3 * SC)

            rbig = stats["rbig"]

            def make_rmst(c):
                t = {}
                for k in range(KT):
                    rm = work.tile([P, SC], MM, name="rms", tag="rms", bufs=8)
                    nc.vector.tensor_mul(rm, hsl(k, c * SC, (c + 1) * SC),
                                         rbig[:, c * SC:(c + 1) * SC])
                    t[k] = rm
                return t

            pending = {}
            rmst = make_rmst(0)
            rmst_next = None
            for c in range(NSC):
                ght = {}
                hgru_c = {}
                for m in [x for k in range(KT) for x in (k, k + KT)]:
                    ps = ps_mm.tile([P, SC], F32, name="psh", tag="psmm")
                    for sub in range(SC // 512):
                        psl = ps[:, sub * 512:(sub + 1) * 512]
                        for k in range(KT):
                            nc.tensor.matmul(
                                psl, w1[li, k][:, m * P:(m + 1) * P],
                                rmst[k][:, sub * 512:(sub + 1) * 512],
                                start=(k == 0), stop=(k == KT - 1))
                    if m < KT:   # hidden half: g = max(hid + 0.5, sigmoid(hid))
                        sg = work.tile([P, SC], SCD, name="sg", tag="sg",
                                       bufs=2)
                        nc.scalar.activation(sg, ps, AF.Sigmoid)
                        gh = work.tile([P, SC], SCD, name="gh", tag="gh",
                                       bufs=2)
                        nc.vector.scalar_tensor_tensor(
                            gh, ps, 0.5, sg, op0=AL.add, op1=AL.max)
                        ght[m] = gh
                        if m == KT - 1 and c < NSC - 1:
                            # prefetch next chunk's rmsnormed tiles on the
                            # DVE ahead of this chunk's last scans, so the
                            # PE's hg(c+1) matmuls never wait on the DVE
                            # draining chunk c
                            rmst_next = make_rmst(c + 1)
                    else:        # gate half: a = sigmoid(-gate); vneg = (a-1)*g
                        k = m - KT
                        at = work.tile([P, SC], SCD, name="a_t", tag="a_t",
                                       bufs=2)
                        nc.scalar.activation(at, ps, AF.Sigmoid, scale=-1.0)
                        vt = work.tile([P, SC], SCD, name="v_t", tag="v_t",
                                       bufs=2)
                        nc.vector.scalar_tensor_tensor(
                            vt, at, 1.0, ght[k], op0=AL.subtract, op1=AL.mult)
                        hgru_c[k] = scan_piece(k, at, vt)
                pending[c] = mk_sl(hgru_c)
                if c == 0:
                    stats["stats_chunk"](3)
                    stats["r_group"](96, 128, 3 * SC, S)
                if c >= 1:
                    out_proj_chunk(li, c - 1, pending.pop(c - 1))
                    if c >= 2:
                        # pipeline the NEXT layer's stats for chunk c-2 (its
                        # h was finalized one chunk ago -- fully data-ready,
                        # so these PE ones-matmuls never stall the queue)
                        if stats_next is not None:
                            stats_next["stats_chunk"](c - 2)
                            if c == NSC - 1:
                                stats_next["stats_chunk"](2)
                        elif c == NSC - 1:
                            # last layer: final-LN stats for half 0
                            ln_stats_half(0)
                rmst = rmst_next
            carry[0] = (li, pending.pop(NSC - 1))

        # ---------------- layer 1 (token-LUT scan inputs) ----------------
        # a/v stream per-k [P, SC] tiles on the sync HWDGE ring (each k's
        # scan starts as soon as its own DMA lands), h0 on the scalar ring
        # (the two rings run in parallel)
        stats2 = None
        l1_pending = {}
        for c in range(NSC):
            if c == 0:
                # chunk 0: interleave (dma a_k, dma v_k, scan k first-half)
                # so each scan's queue-sem threshold covers only its own
                # DMAs; first-half scans in 512-wide separate tiles so the
                # first out-proj sub can start after only the half scans
                halves = {}
                for k in range(KT):
                    c0_dma(k)
                    halves[k, 0] = scan_piece(
                        k, a0t[k][:, 0:512], v0t[k][:, 0:512],
                        width=512, tag="hgruh", bufs=8)
                for k in range(KT):
                    halves[k, 1] = scan_piece(
                        k, a0t[k][:, 512:1024], v0t[k][:, 512:1024],
                        width=512, tag="hgruh", bufs=8)
                l1_pending[0] = lambda k, sub: halves[k, sub][:, :]
                load_early_consts()
                continue
            nc.scalar.dma_start(out=hbig[:, :, c * SC:(c + 1) * SC],
                                in_=d_h0T[c])
            hgru_c = {}
            for k in range(KT):
                apk = work.tile([P, SC], SCD, name="a_pk", tag="c0av",
                                bufs=8)
                vpk = work.tile([P, SC], SCD, name="v_pk", tag="c0av",
                                bufs=8)
                nc.sync.dma_start(out=apk,
                                  in_=d_a1T[c][:, k * SC:(k + 1) * SC])
                nc.sync.dma_start(out=vpk,
                                  in_=d_v1T[c][:, k * SC:(k + 1) * SC])
                hgru_c[k] = scan_piece(k, apk, vpk)
            l1_pending[c] = mk_sl(hgru_c)
            out_proj_chunk(0, c - 1, l1_pending.pop(c - 1))
            if c == 1:
                load_late_consts()
                stats2 = make_rms_stats(1)
            if c >= 2:
                stats2["stats_chunk"](c - 2)
            if c == NSC - 1:
                stats2["stats_chunk"](2)
        carry[0] = (0, l1_pending.pop(NSC - 1))

        stats = stats2
        for li in range(1, L):
            prev_scan = [None] * KT
            stats_next = make_rms_stats(li + 1) if li < L - 1 else None
            mid_layer(li, stats, stats_next)
            stats = stats_next
        pli, phg = carry[0]
        out_proj_chunk(pli, NSC - 1, phg)
        carry[0] = None

        # ---------------- final layernorm + vocab projection ----------------
        # half-0 stats were pipelined into layer L-1's loop; emit its logits
        # now, then half-1 stats + logits. Fences order each half's LN-stat
        # PSUM work before its logits matmuls (shared small PSUM tag).
        tc.no_sync_barrier()
        ln_logits_half(0)
        ln_stats_half(1)
        tc.no_sync_barrier()
        ln_logits_half(1)

    nc.finalize()
    return nc


def _sigmoid(x):
    return 1.0 / (1.0 + np.exp(-x))


def _host_prep(inputs):
    """Precompute per-core device inputs (numpy, float64 internal)."""
    mm_np = _np_dt(MM_KIND)
    sc_np = _np_dt(SC_KIND)

    x = np.asarray(inputs["x"]).astype(np.int64)        # [B, S]
    emb = np.asarray(inputs["emb"]).astype(np.float64)  # [V, D]
    rms_w = np.asarray(inputs["rms_w"]).astype(np.float64)
    W_hg = np.asarray(inputs["W_hg"]).astype(np.float64)
    W_out = np.asarray(inputs["W_out"]).astype(np.float64)
    b_out = np.asarray(inputs["b_out"]).astype(np.float64)
    ln_w = np.asarray(inputs["ln_w"]).astype(np.float64)
    ln_b = np.asarray(inputs["ln_b"]).astype(np.float64)
    out_W = np.asarray(inputs["out_W"]).astype(np.float64)
    out_b = np.asarray(inputs["out_b"]).astype(np.float64)

    # layer-1 token tables
    r0 = 1.0 / np.sqrt((emb * emb).mean(-1, keepdims=True) + EPS_RMS)
    rms1 = emb * r0 * rms_w[0][None, :]
    hg1 = rms1 @ W_hg[0].T
    hid1, gate1 = hg1[:, :D], hg1[:, D:]
    z1 = _sigmoid(gate1)
    a_tab = _sigmoid(-gate1)                               # [V, D]
    # device scans -v and the sign is folded into W_out
    v_tab = -(z1 * np.maximum(hid1 + 0.5, _sigmoid(hid1)))  # [V, D]

    # folded weights (W_out negated: the scan output is -h_gru)
    W1 = np.stack([(W_hg[i] * rms_w[i][None, :]).T for i in range(1, L)])
    W1 = W1.reshape(L - 1, KT, P, 2 * D)
    WoT = np.stack([-W_out[i].T for i in range(L)]).reshape(L, KT, P, D)
    W2 = (out_W * ln_w[None, :]).T                         # [D, V]
    cw = W2.sum(0)[None, :]                                # [1, V]
    b2 = out_b + out_W @ ln_b                              # [V]

    common = {
        "W1": W1.astype(mm_np),
        "WoT": WoT.astype(mm_np),
        "W2": W2.reshape(KT, P, V).astype(mm_np),
        "bout": np.ascontiguousarray(
            b_out.reshape(L, KT, P, 1)).astype(np.float32),
        "b2rep": np.tile(b2[None, :].astype(np.float32), (P, 1)),
        "cwrep": np.tile(cw.astype(np.float32), (P, 1)),
        "onessel": np.tile(np.eye(4).reshape(-1), (P, 1)).astype(mm_np),
        "onesk": np.ones((P, 1), mm_np),
        "onesr": np.ones((1, P), mm_np),
        "ident": np.eye(P, dtype=np.float32).astype(mm_np),
        "identf": np.eye(4, dtype=np.float32),
    }
    def pack(tab_rows):
        # [S, D] token-gathered rows -> [NSC, P, KT*SC] chunk-major pack
        t = tab_rows.T.reshape(KT, P, NSC, SC)        # [k, p, c, j]
        return np.ascontiguousarray(t.transpose(2, 1, 0, 3)
                                    .reshape(NSC, P, KT * SC))

    in_maps = []
    for b in range(B):
        tok = x[b]
        m = dict(common)
        m["h0T"] = pack(emb[tok]).astype(mm_np)
        m["a1T"] = pack(a_tab[tok]).astype(sc_np)
        m["v1T"] = pack(v_tab[tok]).astype(sc_np)
        in_maps.append(m)
    return in_maps


def _get_nc():
    if "nc" not in _cache:
        _cache["nc"] = _build_nc()
    return _cache["nc"]


def kernel(**inputs):
    from concourse.bass_utils import run_bass_kernel_spmd

    nc = _get_nc()
    in_maps = _host_prep(inputs)
    res = run_bass_kernel_spmd(nc, in_maps, core_ids=list(range(N_CORES)),
                               trace=bool(int(os.environ.get("EM_TRACE", "0"))))
    _cache["last_result"] = res
    # device layout [g, p, j*V+v] -> token s = g*1024 + j*128 + p
    out = np.stack([
        np.asarray(r["out"]).reshape(S // (4 * P), P, 4, V)
        .transpose(0, 2, 1, 3).reshape(S, V)
        for r in res.results
    ], axis=0)
    return out.astype(np.float32)


# revision 19
# speedup vs baseline: 1.0865x; 1.0334x over previous
"""Trainium2 Bass kernel for nn_EntropyModel (minGRU LM).

Strategy (8 NeuronCores, data-parallel over batch B=8, one sample per core):

  - Residual stream kept TRANSPOSED on device: hT[d, s] (d on partitions,
    s along free dim), because the minGRU recurrence is computed with the
    DVE `tensor_tensor_scan` instruction (state = a*state + v along the
    free dim, fp32 internal state) which needs lanes on partitions and
    time along free.
  - Layer 1 is token-lookup: rms/hg/nonlinearities of layer 1 depend only
    on the token id (vocab=256), so a1/v1 (scan coefficients/values) and
    h0 (embedding) are precomputed on host as tables and gathered per
    token; the device only runs the scan + out-projection for layer 1.
  - minGRU math:  a = sigmoid(-gate) = 1 - z,  v = z * g(hidden) with
    g(x) = max(x + 0.5, sigmoid(x))  (exact identity for the reference's
    where(x>=0, x+0.5, sigmoid(x))).
    Device computes a directly via sigmoid(-gate) (ACT scale=-1) and
    vneg = (a - 1) * g = -v in ONE scalar_tensor_tensor op; the scan then
    produces -h_gru and the sign is folded into W_out host-side.
  - rmsnorm weight folded into W_hg host-side; layernorm w/b folded into
    the vocab projection: logits = rstd*(W2.T h - mu*colsum(W2)) + b2.
  - b_out rides the out-projection PSUM eviction as the ACT bias operand
    (out = Copy(ps + b_out)); the residual add stays an identity matmul
    inside the PSUM accumulation.
  - Per-token sums over d (rms sumsq, LN mean/meansq) via ones-vector
    matmuls on the PE; per-free broadcast of r[s] via K=1 rank-1 matmuls.
  - Engine balance: all rsqrts use the DVE Newton iteration so the ACT
    sigmoid table stays resident (ACT table reloads cost 1.3us each).
    Bulk weight/h0 DMAs ride the otherwise-idle GpSimd SWDGE ring so
    neither the sync a/v stream nor the ACT queue is obstructed
    (a DMA_DIRECT2D occupies its issuing engine's queue ~0.6us).
    GpSimd compute is avoided: it contends with the DVE SBUF port.
  - Stats for layer li+1 are emitted inside layer li's chunk loop (chunk
    c-2 after out-proj of c-1) so layer boundaries don't stall the PE;
    the final-LN stats for the first half ride inside layer L-1's loop.

Matmul/bf16 dtype knobs below (MM_*: matmul operands, SC_*: scan operands).
"""

import os
import numpy as np
import ml_dtypes

V, D, L, B, S = 256, 512, 4, 8, 4096
EPS_RMS = 1e-5
EPS_LN = 1e-5
P = 128
KT = D // P            # 4 d-tiles of 128
ET = 2 * D // P        # 8 e-tiles for the hidden/gate projection
SC = 1024              # s-chunk for working tiles
NSC = S // SC
N_CORES = 8

# dtype knobs: "bf16" or "f32r" for matmul operands, "bf16"/"f32" for scan
MM_KIND = os.environ.get("EM_MM_KIND", "bf16")
SC_KIND = os.environ.get("EM_SC_KIND", "bf16")

_cache = {}


def _np_dt(kind):
    return np.float32 if kind in ("f32", "f32r") else ml_dtypes.bfloat16


def _build_nc():
    import concourse.bass as bass  # noqa: F401
    import concourse.bacc as bacc
    import concourse.mybir as mybir
    import concourse.tile as tile
    from contextlib import ExitStack

    AL = mybir.AluOpType
    AF = mybir.ActivationFunctionType
    F32 = mybir.dt.float32
    I32 = mybir.dt.int32
    F8 = mybir.dt.float8e4
    MM = {"bf16": mybir.dt.bfloat16, "f32r": mybir.dt.float32r}[MM_KIND]
    SCD = {"bf16": mybir.dt.bfloat16, "f32": mybir.dt.float32}[SC_KIND]

    nc = bacc.Bacc()

    # layer-1 streams packed chunk-major: [c, p, k*SC+j] = orig[k*128+p, c*SC+j]
    # so each chunk is ONE ~1MB DMA (descriptor overhead amortized)
    d_h0T = nc.dram_tensor("h0T", [NSC, P, KT * SC], MM, kind="ExternalInput")
    d_a1T = nc.dram_tensor("a1T", [NSC, P, KT * SC], SCD, kind="ExternalInput")
    d_v1T = nc.dram_tensor("v1T", [NSC, P, KT * SC], SCD, kind="ExternalInput")
    d_W1 = nc.dram_tensor("W1", [L - 1, P, KT, 2 * D], MM, kind="ExternalInput")
    d_WoT = nc.dram_tensor("WoT", [L, P, KT, D], MM, kind="ExternalInput")
    d_W2 = nc.dram_tensor("W2", [P, KT, V], MM, kind="ExternalInput")
    d_bout = nc.dram_tensor("bout", [L, P, KT], F32, kind="ExternalInput")
    d_b2rep = nc.dram_tensor("b2rep", [P, V], F32, kind="ExternalInput")
    d_cwrep = nc.dram_tensor("cwrep", [P, V], F32, kind="ExternalInput")
    d_onessel = nc.dram_tensor("onessel", [P, 16], MM, kind="ExternalInput")
    d_onesk = nc.dram_tensor("onesk", [P, 1], MM, kind="ExternalInput")
    d_onesr = nc.dram_tensor("onesr", [1, P], MM, kind="ExternalInput")
    d_ident = nc.dram_tensor("ident", [P, P], MM, kind="ExternalInput")
    d_identf = nc.dram_tensor("identf", [4, 4], F32, kind="ExternalInput")
    # output laid out [group, partition, 4*V]: token s = g*512 + j*128 + p
    # lives at out[g, p, j*V:(j+1)*V] so a group of 4 token-tiles is one
    # contiguous [128, 1024] DMA; the host un-permutes.
    d_out = nc.dram_tensor("out", [S // (4 * P), P, 4 * V], F32,
                           kind="ExternalOutput")

    with ExitStack() as ctx:
        tc = ctx.enter_context(tile.TileContext(nc))
        consts = ctx.enter_context(tc.tile_pool(name="consts", bufs=1))
        hpool = ctx.enter_context(tc.tile_pool(name="hpool", bufs=1))
        work = ctx.enter_context(tc.tile_pool(name="work", bufs=2))
        small = ctx.enter_context(tc.tile_pool(name="small", bufs=1))
        ps_mm = ctx.enter_context(tc.tile_pool(name="ps_mm", bufs=3, space="PSUM"))
        ps_sm = ctx.enter_context(tc.tile_pool(name="ps_sm", bufs=2, space="PSUM"))

        # ---------------- constants / weights ----------------
        def cdma(name, shape, dt, src, eng=None):
            t = consts.tile(shape, dt, name=name, tag=name)
            (eng or nc.sync).dma_start(out=t, in_=src)
            return t

        # residual stream: ONE [P, KT, S] tile (k-tiles as column blocks) so
        # the per-chunk h0 fill is a single strided DMA; filled per-chunk
        # inside the layer-1 loop so the a1/v1 streams win the DMA queue
        hbig = hpool.tile([P, KT, S], MM, name="hbig", tag="hbig")

        def hsl(k, s0, s1):
            return hbig[:, k, s0:s1]

        eps_sb = consts.tile([P, 1], F32, name="eps", tag="eps")
        nc.vector.memset(eps_sb, EPS_RMS)  # EPS_RMS == EPS_LN

        # chunk-0 a/v per-k tiles: separate tiles AND scan emission
        # interleaved right behind each k's two DMAs.  DMA-completion
        # semaphores are cumulative per queue: a consumer waits for every
        # DMA emitted on that ring before it, so the scan for k must be
        # emitted before k+1's DMAs to start early.  (Scans are emitted in
        # the layer-1 loop below; here we only stage the tiles + DMAs
        # handles so the loop can interleave.)
        a0t, v0t = [], []

        def c0_dma(k):
            at0 = work.tile([P, SC], SCD, name=f"a0_{k}", tag="c0av", bufs=8)
            vt0 = work.tile([P, SC], SCD, name=f"v0_{k}", tag="c0av", bufs=8)
            nc.sync.dma_start(out=at0, in_=d_a1T[0][:, k * SC:(k + 1) * SC])
            nc.sync.dma_start(out=vt0, in_=d_v1T[0][:, k * SC:(k + 1) * SC])
            a0t.append(at0)
            v0t.append(vt0)

        # h0 + weights ride the gpsimd SWDGE ring: the scalar queue must
        # stay free for ACT compute (a DMA_DIRECT2D occupies the issuing
        # engine ~0.6us), and the sync ring is reserved for the a/v streams
        nc.gpsimd.dma_start(out=hbig[:, :, 0:SC], in_=d_h0T[0])
        ident_sb = cdma("ident", [P, P], MM, d_ident[:, :], nc.gpsimd)
        bout_sb = {}
        bout_sb[0] = cdma("bout_0", [P, KT], F32, d_bout[0], nc.gpsimd)
        wot = {}
        wot[0] = cdma("wot_0", [P, KT, D], MM, d_WoT[0], nc.gpsimd)

        def load_early_consts():
            # tiny ones-vectors: needed by layer-2 stats that overlap L1
            nonlocal onesk_sb, onesr_sb
            onesk_sb = cdma("onesk", [P, 1], MM, d_onesk[:, :])
            onesr_sb = cdma("onesr", [1, P], MM, d_onesr[:, :])

        def load_late_consts():
            # everything on the gpsimd SWDGE ring (idle engine, keeps both
            # the sync a/v stream and the ACT queue unobstructed), one
            # packed DMA per layer/tensor, ordered by first use
            nonlocal cwrep_sb, onessel_sb, b2_sb, w1, w2sb, identf_sb
            for li in range(1, L):
                w1[li] = cdma(f"w1_{li}", [P, KT, 2 * D], MM,
                              d_W1[li - 1], nc.gpsimd)
                wot[li] = cdma(f"wot_{li}", [P, KT, D], MM, d_WoT[li],
                               nc.gpsimd)
                bout_sb[li] = cdma(f"bout_{li}", [P, KT], F32, d_bout[li],
                                   nc.gpsimd)
            w2sb = cdma("w2", [P, KT, V], MM, d_W2[:, :, :], nc.gpsimd)
            b2_sb = cdma("b2", [P, V], F32, d_b2rep[:, :], nc.gpsimd)
            cwrep_sb = cdma("cwrep", [P, V], F32, d_cwrep[:, :], nc.gpsimd)
            onessel_sb = cdma("onessel", [P, 16], MM, d_onessel[:, :],
                              nc.gpsimd)
            identf_sb = cdma("identf", [4, 4], F32, d_identf[:, :],
                             nc.gpsimd)

        onesk_sb = onesr_sb = cwrep_sb = onessel_sb = b2_sb = w2sb = None
        identf_sb = None
        w1 = {}

        # ---------------- helpers ----------------
        # NOTE on emission order: the Tile scheduler pops data-ready
        # instructions per engine in emission order and COMMITS them to the
        # engine FIFO even when their tile-slot is still occupied. So a tile
        # allocation must never depend (through slot reuse) on same-engine
        # work emitted later. Everything below is therefore emitted fully
        # chunk-interleaved: produce (a, v) for chunk c, scan chunk c, run
        # the output projection for chunk c, then move to chunk c+1.
        prev_scan = [None] * KT   # (tile, width)

        def scan_piece(k, at, vt, width=SC, tag="hgru", bufs=8):
            hg_t = work.tile([P, width], MM, name="hgru", tag=tag, bufs=bufs)
            if prev_scan[k] is None:
                init = 0.0
            else:
                pt, pw = prev_scan[k]
                init = pt[:, pw - 1:pw]
            nc.vector.tensor_tensor_scan(
                hg_t, at, vt, init, op0=AL.mult, op1=AL.add)
            prev_scan[k] = (hg_t, width)
            return hg_t

        def out_proj_chunk(li, c, hg_sl):
            """h[:, chunk c] += W_out.T @ hgru + b_out (resid via identity MM,
            bias via the ACT eviction).  hg_sl(k, sub) -> [P, 512] AP."""
            for m in range(KT):
                ps = ps_mm.tile([P, SC], F32, name="pso", tag="psmm")
                for sub in range(SC // 512):
                    psl = ps[:, sub * 512:(sub + 1) * 512]
                    base = c * SC + sub * 512
                    for k in range(KT):
                        nc.tensor.matmul(
                            psl, wot[li][:, k, m * P:(m + 1) * P],
                            hg_sl(k, sub), start=(k == 0), stop=False)
                    nc.tensor.matmul(psl, ident_sb, hsl(m, base, base + 512),
                                     start=False, stop=True)
                nc.scalar.activation(hsl(m, c * SC, (c + 1) * SC), ps,
                                     AF.Identity, bias=bout_sb[li][:, m:m + 1])

        def mk_sl(hgru_c):
            return lambda k, sub: hgru_c[k][:, sub * 512:(sub + 1) * 512]

        def rsqrt_rows(dst, src, p0, p1, scale, eps, tagp):
            """dst[p0:p1] = 1/sqrt(src[p0:p1]*scale + eps) via DVE Newton
            (seed + 2 NR iters) -- keeps the ACT sigmoid table resident."""
            W = src.shape[-1]
            fm = small.tile([P, W], F32, name="fm" + tagp, tag="fm" + tagp,
                            bufs=2)
            fa = small.tile([P, W], F32, name="fa" + tagp, tag="fa" + tagp,
                            bufs=2)
            fb = small.tile([P, W], F32, name="fb" + tagp, tag="fb" + tagp,
                            bufs=2)
            fc = small.tile([P, W], F32, name="fc" + tagp, tag="fc" + tagp,
                            bufs=2)
            sl = (slice(p0, p1), slice(None))
            nc.vector.tensor_scalar(fm[sl], src[sl], scale, eps,
                                    op0=AL.mult, op1=AL.add)
            nc.vector.tensor_scalar(fa.bitcast(I32)[sl], fm.bitcast(I32)[sl],
                                    1, None, op0=AL.logical_shift_right)
            nc.vector.tensor_scalar(fb.bitcast(I32)[sl], fa.bitcast(I32)[sl],
                                    -1, 0x5F3759DF, op0=AL.mult, op1=AL.add)
            for _ in range(2):
                nc.vector.tensor_mul(fa[sl], fb[sl], fb[sl])
                nc.vector.tensor_mul(fc[sl], fa[sl], fm[sl])
                nc.vector.tensor_scalar(fa[sl], fc[sl], -0.5, 1.5,
                                        op0=AL.mult, op1=AL.add)
                nc.vector.tensor_mul(fb[sl], fb[sl], fa[sl])
            nc.vector.tensor_copy(dst[sl], fb[sl])

        # -------- rms stats machinery (pipelined across layers) --------
        def make_rms_stats(li):
            st = {}
            st["rt_raw"] = small.tile([P, S // P], F32, name="rtraw",
                                      tag="rtraw", bufs=2)
            st["rrmm"] = small.tile([P, S // P], MM, name="rrmm", tag="rrmm",
                                    bufs=2)
            st["r_row"] = small.tile([1, S], MM, name="rrow", tag="rrow",
                                     bufs=1)
            st["rbig"] = work.tile([P, S], MM, name="rbig", tag="rbig",
                                   bufs=1)

            def stats_chunk(c):
                # squares split ACT/DVE (gpsimd would contend for the DVE
                # SBUF port and slow every scan/STT by ~15%)
                sqt = {}
                for k in range(KT):
                    sq = work.tile([P, SC], MM, name="sq", tag="sq", bufs=6)
                    if k % 2 == 0:
                        nc.scalar.activation(
                            sq, hsl(k, c * SC, (c + 1) * SC), AF.Square)
                    else:
                        nc.vector.tensor_mul(sq, hsl(k, c * SC, (c + 1) * SC),
                                             hsl(k, c * SC, (c + 1) * SC))
                    sqt[k] = sq
                for sub in range(SC // 512):
                    n = c * (SC // 512) + sub
                    ps = ps_sm.tile([1, 512], F32, name="pstat", tag="small")
                    for k in range(KT):
                        nc.tensor.matmul(
                            ps, onesk_sb, sqt[k][:, sub * 512:(sub + 1) * 512],
                            start=(k == 0), stop=(k == KT - 1))
                    stg = small.tile([1, 512], F32, name="stg", tag="stage",
                                     bufs=3)
                    nc.scalar.copy(out=stg, in_=ps)
                    nc.scalar.dma_start(
                        out=st["rt_raw"][16 * n:16 * (n + 1), :], in_=stg)

            def r_group(p0, p1, s0, s1):
                # r = 1/sqrt(sumsq/D + eps) for stat rows [p0,p1) = tokens
                # [s0,s1), then row-ify and broadcast across partitions
                rsqrt_rows(st["rrmm"], st["rt_raw"], p0, p1, 1.0 / D, EPS_RMS,
                           "r")
                nc.scalar.dma_start(out=st["r_row"][:, s0:s1],
                                    in_=st["rrmm"][p0:p1, :])
                for n in range(s0 // 512, s1 // 512):
                    psb = ps_sm.tile([P, 512], F32, name="psb", tag="small")
                    nc.tensor.matmul(psb, onesr_sb,
                                     st["r_row"][:, n * 512:(n + 1) * 512],
                                     start=True, stop=True)
                    nc.scalar.copy(out=st["rbig"][:, n * 512:(n + 1) * 512],
                                   in_=psb)

            st["stats_chunk"] = stats_chunk
            st["r_group"] = r_group
            return st

        # -------- final-LN stats machinery (two halves) --------
        ln = {"mn": None, "nrstd": None}

        def ln_stats_half(half):
            sqf = {}
            for c in range(half * NSC // 2, (half + 1) * NSC // 2):
                for k in range(KT):
                    sq = work.tile([P, SC], MM, name="sqf", tag="sq", bufs=6)
                    if k % 2 == 0:
                        nc.scalar.activation(
                            sq, hsl(k, c * SC, (c + 1) * SC), AF.Square)
                    else:
                        nc.vector.tensor_mul(sq, hsl(k, c * SC, (c + 1) * SC),
                                             hsl(k, c * SC, (c + 1) * SC))
                    sqf[k, c] = sq
            # Per-token stats: ones-matmuls write 4 sequence-subs into the 4
            # partition-rows of ONE PSUM [4, 512] tile (one-hot column
            # weights, one long accumulation chain), evicted once and flipped
            # to token-on-partition [P, 16] via PE transposes. Column layout
            # j-major: col = j*4 + n_local.
            psg_m = ps_sm.tile([4, 512], F32, name="psgm", tag="small")
            psg_q = ps_sm.tile([4, 512], F32, name="psgq", tag="small")
            nmm = 0
            for nl, n in enumerate(range(half * 4, half * 4 + 4)):
                for k in range(KT):
                    nc.tensor.matmul(psg_m, onessel_sb[:, nl * 4:(nl + 1) * 4],
                                     hsl(k, n * 512, (n + 1) * 512),
                                     start=(nmm == 0), stop=(nmm == 15))
                    nmm += 1
            nmm = 0
            for nl, n in enumerate(range(half * 4, half * 4 + 4)):
                c, sub = divmod(n, SC // 512)
                for k in range(KT):
                    nc.tensor.matmul(psg_q, onessel_sb[:, nl * 4:(nl + 1) * 4],
                                     sqf[k, c][:, sub * 512:(sub + 1) * 512],
                                     start=(nmm == 0), stop=(nmm == 15))
                    nmm += 1
            stage_m = small.tile([4, 512], F32, name="stgm", tag="stgm",
                                 bufs=2)
            stage_q = small.tile([4, 512], F32, name="stgq", tag="stgq",
                                 bufs=2)
            nc.scalar.copy(out=stage_m, in_=psg_m)
            nc.scalar.copy(out=stage_q, in_=psg_q)
            HC = 16                        # stat columns per half
            mth = small.tile([P, HC], F32, name="mth", tag="mth", bufs=2)
            qth = small.tile([P, HC], F32, name="qth", tag="qth", bufs=2)
            for j in range(4):
                pst = ps_sm.tile([P, 4], F32, name="pst", tag="small")
                nc.tensor.transpose(pst, stage_m[0:4, j * P:(j + 1) * P],
                                    identf_sb[:, :])
                nc.scalar.copy(out=mth[:, j * 4:(j + 1) * 4], in_=pst)
                psu = ps_sm.tile([P, 4], F32, name="psu", tag="small")
                nc.tensor.transpose(psu, stage_q[0:4, j * P:(j + 1) * P],
                                    identf_sb[:, :])
                nc.scalar.copy(out=qth[:, j * 4:(j + 1) * 4], in_=psu)
            # rstd = 1/sqrt(E[h^2] - E[h]^2 + eps) for this half's columns
            mn = small.tile([P, HC], F32, name="mn", tag="mn", bufs=2)
            nc.vector.tensor_scalar_mul(mn, mth, 1.0 / D)
            qn = small.tile([P, HC], F32, name="qn", tag="qn", bufs=2)
            nc.vector.tensor_scalar_mul(qn, qth, 1.0 / D)
            msq = small.tile([P, HC], F32, name="msq", tag="msq", bufs=2)
            nc.vector.tensor_mul(msq, mn, mn)
            var = small.tile([P, HC], F32, name="var", tag="var", bufs=2)
            nc.vector.tensor_sub(var, qn, msq)
            rstd = small.tile([P, HC], F32, name="rstd", tag="rstd", bufs=2)
            rsqrt_rows(rstd, var, 0, P, 1.0, EPS_LN, "l")
            nrstd = small.tile([P, HC], F32, name="nrstd", tag="nrstd",
                               bufs=2)
            nc.vector.tensor_scalar_mul(nrstd, rstd, -1.0)
            ln["mn"], ln["nrstd"] = mn, nrstd

        def ln_logits_half(half):
            # logits[s,v] = rstd[s]*((W2.T h)[s,v] - mu[s]*cw[v]) + b2[v]
            #             = (mu*cw - psl) * (-rstd) + b2
            # PSUM evicted to SBUF on ACT; corrections on GpSimd (frees DVE)
            mn, nrstd = ln["mn"], ln["nrstd"]
            ostage = None
            SH = S // 2
            for sti in range(SH // P):
                st = half * (SH // P) + sti
                psl = ps_sm.tile([P, V], F32, name="psl", tag="small")
                for k in range(KT):
                    nc.tensor.matmul(psl, hsl(k, st * P, (st + 1) * P),
                                     w2sb[:, k, :], start=(k == 0),
                                     stop=(k == KT - 1))
                g, j = divmod(st, 4)
                rcol = (sti % 4) * 4 + sti // 4
                if j == 0:
                    ostage = small.tile([P, 4 * V], F32, name="ostage",
                                        tag="ostage", bufs=2)
                t1 = work.tile([P, V], F32, name="t1", tag="t1", bufs=2)
                nc.vector.scalar_tensor_tensor(
                    t1, cwrep_sb, mn[:, rcol:rcol + 1], psl,
                    op0=AL.mult, op1=AL.subtract)
                nc.vector.scalar_tensor_tensor(
                    ostage[:, j * V:(j + 1) * V], t1, nrstd[:, rcol:rcol + 1],
                    b2_sb, op0=AL.mult, op1=AL.add)
                if j == 3:
                    nc.sync.dma_start(out=d_out[g], in_=ostage)

        carry = [None]   # (li, hg_sl) of the previous layer's last chunk

        def mid_layer(li, stats, stats_next):
            """rmsnorm -> hidden/gate -> (a,v) -> scan -> out-proj, chunked.
            `stats` (this layer's rms stats) already has chunks 0-1 emitted;
            `stats_next` is the NEXT layer's ctx to pipeline into this loop
            (None for the last layer: final-LN stats are emitted instead)."""
            if carry[0] is not None:
                # previous layer's last out-proj first: fills the PE while
                # this layer's stats chain gets going
                pli, psl_ = carry[0]
                out_proj_chunk(pli, NSC - 1, psl_)
                carry[0] = None
            # stats chunks 0-2 were pipelined into the previous layer's
            # loop; r for tokens of chunks 0-2 is ready immediately.  Chunk
            # 3's stats (which need the carry out-proj's eviction) are
            # emitted inside the c==0 iteration so their PE/gpsimd work
            # never head-of-line-blocks the first hg matmuls.
            stats["r_group"](0, 96, 0, 3 * SC)

            rbig = stats["rbig"]

            def make_rmst(c):
                t = {}
                for k in range(KT):
                    rm = work.tile([P, SC], MM, name="rms", tag="rms", bufs=8)
                    nc.vector.tensor_mul(rm, hsl(k, c * SC, (c + 1) * SC),
                                         rbig[:, c * SC:(c + 1) * SC])
                    t[k] = rm
                return t

            pending = {}
            rmst = make_rmst(0)
            rmst_next = None
            for c in range(NSC):
                ght = {}
                hgru_c = {}
                for m in [x for k in range(KT) for x in (k, k + KT)]:
                    ps = ps_mm.tile([P, SC], F32, name="psh", tag="psmm")
                    for sub in range(SC // 512):
                        psl = ps[:, sub * 512:(sub + 1) * 512]
                        for k in range(KT):
                            nc.tensor.matmul(
                                psl, w1[li][:, k, m * P:(m + 1) * P],
                                rmst[k][:, sub * 512:(sub + 1) * 512],
                                start=(k == 0), stop=(k == KT - 1))
                    if m < KT:   # hidden half: g = max(hid + 0.5, sigmoid(hid))
                        sg = work.tile([P, SC], SCD, name="sg", tag="sg",
                                       bufs=2)
                        nc.scalar.activation(sg, ps, AF.Sigmoid)
                        gh = work.tile([P, SC], SCD, name="gh", tag="gh",
                                       bufs=2)
                        nc.vector.scalar_tensor_tensor(
                            gh, ps, 0.5, sg, op0=AL.add, op1=AL.max)
                        ght[m] = gh
                        if m == KT - 1 and c < NSC - 1:
                            # prefetch next chunk's rmsnormed tiles on the
                            # DVE ahead of this chunk's last scans, so the
                            # PE's hg(c+1) matmuls never wait on the DVE
                            # draining chunk c
                            rmst_next = make_rmst(c + 1)
                    else:        # gate half: a = sigmoid(-gate); vneg = (a-1)*g
                        k = m - KT
                        at = work.tile([P, SC], SCD, name="a_t", tag="a_t",
                                       bufs=2)
                        nc.scalar.activation(at, ps, AF.Sigmoid, scale=-1.0)
                        vt = work.tile([P, SC], SCD, name="v_t", tag="v_t",
                                       bufs=2)
                        nc.vector.scalar_tensor_tensor(
                            vt, at, 1.0, ght[k], op0=AL.subtract, op1=AL.mult)
                        hgru_c[k] = scan_piece(k, at, vt)
                pending[c] = mk_sl(hgru_c)
                if c == 0:
                    stats["stats_chunk"](3)
                    stats["r_group"](96, 128, 3 * SC, S)
                if c >= 1:
                    out_proj_chunk(li, c - 1, pending.pop(c - 1))
                    if c >= 2:
                        # pipeline the NEXT layer's stats for chunk c-2 (its
                        # h was finalized one chunk ago -- fully data-ready,
                        # so these PE ones-matmuls never stall the queue)
                        if stats_next is not None:
                            stats_next["stats_chunk"](c - 2)
                            if c == NSC - 1:
                                stats_next["stats_chunk"](2)
                        elif c == NSC - 1:
                            # last layer: final-LN stats for half 0
                            ln_stats_half(0)
                rmst = rmst_next
            carry[0] = (li, pending.pop(NSC - 1))

        # ---------------- layer 1 (token-LUT scan inputs) ----------------
        # a/v stream per-k [P, SC] tiles on the sync HWDGE ring (each k's
        # scan starts as soon as its own DMA lands), h0 on the scalar ring
        # (the two rings run in parallel)
        stats2 = None
        l1_pending = {}
        for c in range(NSC):
            if c == 0:
                # chunk 0: interleave (dma a_k, dma v_k, scan k first-half)
                # so each scan's queue-sem threshold covers only its own
                # DMAs; first-half scans in 512-wide separate tiles so the
                # first out-proj sub can start after only the half scans
                halves = {}
                for k in range(KT):
                    c0_dma(k)
                    halves[k, 0] = scan_piece(
                        k, a0t[k][:, 0:512], v0t[k][:, 0:512],
                        width=512, tag="hgruh", bufs=8)
                for k in range(KT):
                    halves[k, 1] = scan_piece(
                        k, a0t[k][:, 512:1024], v0t[k][:, 512:1024],
                        width=512, tag="hgruh", bufs=8)
                l1_pending[0] = lambda k, sub: halves[k, sub][:, :]
                load_early_consts()
                continue
            nc.gpsimd.dma_start(out=hbig[:, :, c * SC:(c + 1) * SC],
                                in_=d_h0T[c])
            hgru_c = {}
            for k in range(KT):
                apk = work.tile([P, SC], SCD, name="a_pk", tag="c0av",
                                bufs=8)
                vpk = work.tile([P, SC], SCD, name="v_pk", tag="c0av",
                                bufs=8)
                nc.sync.dma_start(out=apk,
                                  in_=d_a1T[c][:, k * SC:(k + 1) * SC])
                nc.sync.dma_start(out=vpk,
                                  in_=d_v1T[c][:, k * SC:(k + 1) * SC])
                hgru_c[k] = scan_piece(k, apk, vpk)
            l1_pending[c] = mk_sl(hgru_c)
            out_proj_chunk(0, c - 1, l1_pending.pop(c - 1))
            if c == 1:
                load_late_consts()
                stats2 = make_rms_stats(1)
            if c >= 2:
                stats2["stats_chunk"](c - 2)
            if c == NSC - 1:
                stats2["stats_chunk"](2)
        carry[0] = (0, l1_pending.pop(NSC - 1))

        stats = stats2
        for li in range(1, L):
            prev_scan = [None] * KT
            stats_next = make_rms_stats(li + 1) if li < L - 1 else None
            mid_layer(li, stats, stats_next)
            stats = stats_next
        pli, phg = carry[0]
        out_proj_chunk(pli, NSC - 1, phg)
        carry[0] = None

        # ---------------- final layernorm + vocab projection ----------------
        # half-0 stats were pipelined into layer L-1's loop; emit its logits
        # now, then half-1 stats + logits. Fences order each half's LN-stat
        # PSUM work before its logits matmuls (shared small PSUM tag).
        tc.no_sync_barrier()
        ln_logits_half(0)
        ln_stats_half(1)
        tc.no_sync_barrier()
        ln_logits_half(1)

    nc.finalize()
    return nc


def _sigmoid(x):
    return 1.0 / (1.0 + np.exp(-x))


def _host_prep(inputs):
    """Precompute per-core device inputs (numpy, float64 internal)."""
    mm_np = _np_dt(MM_KIND)
    sc_np = _np_dt(SC_KIND)

    x = np.asarray(inputs["x"]).astype(np.int64)        # [B, S]
    emb = np.asarray(inputs["emb"]).astype(np.float64)  # [V, D]
    rms_w = np.asarray(inputs["rms_w"]).astype(np.float64)
    W_hg = np.asarray(inputs["W_hg"]).astype(np.float64)
    W_out = np.asarray(inputs["W_out"]).astype(np.float64)
    b_out = np.asarray(inputs["b_out"]).astype(np.float64)
    ln_w = np.asarray(inputs["ln_w"]).astype(np.float64)
    ln_b = np.asarray(inputs["ln_b"]).astype(np.float64)
    out_W = np.asarray(inputs["out_W"]).astype(np.float64)
    out_b = np.asarray(inputs["out_b"]).astype(np.float64)

    # layer-1 token tables
    r0 = 1.0 / np.sqrt((emb * emb).mean(-1, keepdims=True) + EPS_RMS)
    rms1 = emb * r0 * rms_w[0][None, :]
    hg1 = rms1 @ W_hg[0].T
    hid1, gate1 = hg1[:, :D], hg1[:, D:]
    z1 = _sigmoid(gate1)
    a_tab = _sigmoid(-gate1)                               # [V, D]
    # device scans -v and the sign is folded into W_out
    v_tab = -(z1 * np.maximum(hid1 + 0.5, _sigmoid(hid1)))  # [V, D]

    # folded weights (W_out negated: the scan output is -h_gru)
    W1 = np.stack([(W_hg[i] * rms_w[i][None, :]).T for i in range(1, L)])
    W1 = np.ascontiguousarray(
        W1.reshape(L - 1, KT, P, 2 * D).transpose(0, 2, 1, 3))
    WoT = np.ascontiguousarray(
        np.stack([-W_out[i].T for i in range(L)])
        .reshape(L, KT, P, D).transpose(0, 2, 1, 3))
    W2 = (out_W * ln_w[None, :]).T                         # [D, V]
    cw = W2.sum(0)[None, :]                                # [1, V]
    b2 = out_b + out_W @ ln_b                              # [V]

    common = {
        "W1": W1.astype(mm_np),
        "WoT": WoT.astype(mm_np),
        "W2": np.ascontiguousarray(
            W2.reshape(KT, P, V).transpose(1, 0, 2)).astype(mm_np),
        "bout": np.ascontiguousarray(
            b_out.reshape(L, KT, P).transpose(0, 2, 1)).astype(np.float32),
        "b2rep": np.tile(b2[None, :].astype(np.float32), (P, 1)),
        "cwrep": np.tile(cw.astype(np.float32), (P, 1)),
        "onessel": np.tile(np.eye(4).reshape(-1), (P, 1)).astype(mm_np),
        "onesk": np.ones((P, 1), mm_np),
        "onesr": np.ones((1, P), mm_np),
        "ident": np.eye(P, dtype=np.float32).astype(mm_np),
        "identf": np.eye(4, dtype=np.float32),
    }
    def pack(tab_rows):
        # [S, D] token-gathered rows -> [NSC, P, KT*SC] chunk-major pack
        t = tab_rows.T.reshape(KT, P, NSC, SC)        # [k, p, c, j]
        return np.ascontiguousarray(t.transpose(2, 1, 0, 3)
                                    .reshape(NSC, P, KT * SC))

    in_maps = []
    for b in range(B):
        tok = x[b]
        m = dict(common)
        m["h0T"] = pack(emb[tok]).astype(mm_np)
        m["a1T"] = pack(a_tab[tok]).astype(sc_np)
        m["v1T"] = pack(v_tab[tok]).astype(sc_np)
        in_maps.append(m)
    return in_maps


def _get_nc():
    if "nc" not in _cache:
        _cache["nc"] = _build_nc()
    return _cache["nc"]


def kernel(**inputs):
    from concourse.bass_utils import run_bass_kernel_spmd

    nc = _get_nc()
    in_maps = _host_prep(inputs)
    res = run_bass_kernel_spmd(nc, in_maps, core_ids=list(range(N_CORES)),
                               trace=bool(int(os.environ.get("EM_TRACE", "0"))))
    _cache["last_result"] = res
    # device layout [g, p, j*V+v] -> token s = g*1024 + j*128 + p
    out = np.stack([
        np.asarray(r["out"]).reshape(S // (4 * P), P, 4, V)
        .transpose(0, 2, 1, 3).reshape(S, V)
        for r in res.results
    ], axis=0)
    return out.astype(np.float32)
